# revision 1
# baseline (speedup 1.0000x reference)
"""Trainium2 Bass kernel for nn_DeformableFusion.

Pipeline (reference): concat(ft1,ft2,ft3) -> conv3x3(768->256)+relu ->
conv3x3(256->256)+relu -> conv3x3(256->36) = offsets -> two deformable
convs (ft_2, ft_3) -> concat(ft_1, a2, a3) -> conv1x1(768->256).

Sharding: 8 cores = 2 samples x 4 row-strips of 32 rows. Each core gets
zero-padded strip inputs for the conv chain plus full pixel-major copies
of ft_2/ft_3 so the deformable gather needs no halo exchange.

All matmuls run as float32r (full-rate fp32). Convs use the flat
shift-accumulation trick on a width-134 zero-padded grid. The bilinear
gather uses indirect DMA (1KB per pixel: all 256 channels of one source
pixel), combined with per-partition scalar weights, then PE-transposed
back to channel-major for the deformable matmul.
"""
import sys
from contextlib import ExitStack

sys.path.insert(0, "/opt/trn_rl_repo")

import numpy as np

import concourse.bass as bass
import concourse.mybir as mybir
from concourse.bass import IndirectOffsetOnAxis
from concourse.bass_utils import run_bass_kernel_spmd
from concourse.tile import TileContext

f32 = mybir.dt.float32
f32r = mybir.dt.float32  # float32r is ~TF32 precision on HW (rel 1.6e-3); keep exact fp32
i32 = mybir.dt.int32
AF = mybir.ActivationFunctionType
ALU = mybir.AluOpType

P = 128
B, C, H, W = 2, 256, 128, 128
KK = 9
NCORES = 8
SR = 32          # strip rows per core
WP = 134         # padded grid width (x in [-3, 131))
XOFF = 3         # image x -> padded col offset
NB = 402         # conv matmul moving-block size (3 rows of WP)
HW = H * W
MAGIC = 12582912.0  # 1.5 * 2**23, fp32 round-to-int magic


# --------------------------------------------------------------------------
# Walrus in this toolchain rejects instructions carrying more than ~2 sync
# waits ("Too many sync wait commands" on the Tile tail Drain). Spread the
# global-clock waits one-per-NOP before the drain.
# --------------------------------------------------------------------------
def _patch_tile_drain():
    import re

    import bass_rust
    import concourse.tile as tile_mod

    ScopedClock = bass_rust.ScopedClock
    VectorClock = bass_rust.VectorClock

    def _vc_ticks(vc):
        m = re.search(r"VectorClock\(\[(.*)\]\)", repr(vc))
        body = m.group(1).strip()
        return [int(t) for t in body.split(",")] if body else []

    def _drain_and_barrier(self, tick_clock, wait_clock):
        ticks = _vc_ticks(tick_clock.global_clock)
        for proc, tick in enumerate(ticks):
            if tick <= 0:
                continue
            single = [0] * len(ticks)
            single[proc] = tick
            nop = self.nc.sync.nop(nofuse=True, hint=f"drain_wait_p{proc}")
            wait_clock.add_sem_waits(
                nop.ins, ScopedClock({None: VectorClock(single)})
            )
        drain_inst = self.nc.sync.drain()
        wait_clock.add_sem_waits(
            drain_inst.ins,
            ScopedClock({None: tick_clock.global_clock}),
            ScopedClock({None: tick_clock.global_clock.copy()}),
        )
        self.nc.all_engine_barrier()
        assert self.sems is not None
        popped = self.nc._tile_sem_poison_stack.pop()
        assert popped is self._sem_poison
        self.nc.clear_and_free_semaphores(list(self.sems.allocated().values()))
        self.nc.all_engine_barrier()

    tile_mod.TileContext._drain_and_barrier = _drain_and_barrier


_patch_tile_drain()


def _split_sync_waits(nc, cap=1):
    """Walrus in this toolchain caps sync waits per instruction. Hoist
    excess waits onto same-engine NoOps inserted immediately before the
    overloaded instruction (engines are in-order, so waiting earlier on
    the same engine is always safe in this straight-line program)."""
    n = 0
    for bb in nc.m.functions[0].blocks:
        insts = bb.instructions
        i = 0
        while i < len(insts):
            inst = insts[i]
            si = inst.sync_info
            waits = si.on_wait if si is not None else None
            if waits and len(waits) > cap:
                excess = waits[cap:]
                del waits[cap:]
                for j in range(0, len(excess), cap):
                    nop = mybir.InstNoOp(
                        name=f"I-waitsplit-{n}", ins=[], outs=[],
                        engine=inst.engine,
                        sync_info=mybir.SyncInfo(
                            on_wait=excess[j:j + cap], on_update=[]),
                        bass_nofuse=True,
                    )
                    n += 1
                    insts.insert(i, nop)
                    i += 1
            i += 1
    return n


# --------------------------------------------------------------------------
# Device program
# --------------------------------------------------------------------------
def _r(ap):
    return ap.bitcast(f32r)


def _conv_pass(nc, ppool, in_tiles, w_sb, n_cc, n_oj, taps,
               rows_out, out_write):
    """Shift-accumulation 3x3 conv over the flat padded grid.

    in_tiles: per-cc SBUF tiles holding rows_out+2 rows at flat offset 1.
    w_sb: [P, n_cc*9*n_oj*M] weight tile, M = out partition size.
    out_write(oj, p0, size, psum_ap): consume one accumulated block.
    """
    total = rows_out * WP
    m = w_sb.shape[-1]
    p0 = 0
    while p0 < total:
        size = min(NB, total - p0)
        for oj in range(n_oj):
            ps = ppool.tile([P, 512], f32, tag="cpsum", name="psc")
            nmm = n_cc * taps
            i = 0
            for cc in range(n_cc):
                for t in range(taps):
                    ty, tx = t // 3, t % 3
                    roff = 1 + p0 + ty * WP + (tx - 1)
                    nc.tensor.matmul(
                        ps[:m, :size],
                        w_sb[:, ((cc * taps + t) * n_oj + oj), :],
                        in_tiles[cc][:, roff:roff + size],
                        start=(i == 0),
                        stop=(i == nmm - 1),
                    )
                    i += 1
            out_write(oj, p0, size, ps)
        p0 += size


def build_program(split_waits=True):
    nc = bass.Bass("TRN2", target_bir_lowering=False, debug=False,
                   num_devices=NCORES)

    def din(name, shape, dtype=f32):
        return nc.dram_tensor(name, shape, dtype, kind="ExternalInput").ap()

    # conv-chain input strip: 38 rows x 134 cols, zero-padded, 6 c-chunks
    xin = din("xin", [6, P, 38 * WP + 2])
    ft1s = din("ft1s", [2, P, SR * W])          # fuse input strip
    ftp = [din("ftp2", [B * HW, C]), din("ftp3", [B * HW, C])]
    w1l = din("w1l", [P, 6 * 9 * 2, P])
    w2l = din("w2l", [P, 2 * 9 * 2, P])
    w3l = din("w3l", [P, 2 * 9 * 1, 50])
    wdl = [din("wd2l", [P, 9 * 2 * 2, P]), din("wd3l", [P, 9 * 2 * 2, P])]
    wfl = din("wfl", [P, 6 * 2, P])
    b1 = din("b1", [P, 2])
    b2 = din("b2", [P, 2])
    b3 = din("b3", [50, 1])
    bf = din("bf", [P, 2])
    kyrow = din("kyrow", [P, 36])   # (j,k): j + ky[k]
    xkx = din("xkx", [P, 36])       # (j,k): x(part) + kx[k]
    meta = din("meta", [P, 2])      # col0: b*HW (idx offset), col1: r0
    m1m = din("m1m", [P, 36 * WP])  # h1 image-boundary mask (rows+cols)
    m2m = din("m2m", [P, 34 * WP])  # h2 image-boundary mask
    ident = din("ident", [P, P])
    out = nc.dram_tensor("out", [2, P, SR * W], f32, kind="ExternalOutput").ap()

    with TileContext(nc) as tc, ExitStack() as es:
        cst = es.enter_context(tc.tile_pool(name="cst", bufs=1))
        ky_sb = cst.tile([P, 36], f32)
        nc.sync.dma_start(out=ky_sb[:], in_=kyrow[:])
        xk_sb = cst.tile([P, 36], f32)
        nc.sync.dma_start(out=xk_sb[:], in_=xkx[:])
        mt_sb = cst.tile([P, 2], f32)
        nc.sync.dma_start(out=mt_sb[:], in_=meta[:])
        id_sb = cst.tile([P, P], f32)
        nc.sync.dma_start(out=id_sb[:], in_=ident[:])
        b1_sb = cst.tile([P, 2], f32)
        nc.sync.dma_start(out=b1_sb[:], in_=b1[:])
        b2_sb = cst.tile([P, 2], f32)
        nc.sync.dma_start(out=b2_sb[:], in_=b2[:])
        b3_sb = cst.tile([50, 1], f32)
        nc.sync.dma_start(out=b3_sb[:], in_=b3[:])
        bf_sb = cst.tile([P, 2], f32)
        nc.sync.dma_start(out=bf_sb[:], in_=bf[:])

        p_off = es.enter_context(tc.tile_pool(name="p_off", bufs=1))
        off = p_off.tile([50, 32 * WP], f32)

        # ============== conv chain (h1/h2 live only here) ==============
        with tc.tile_pool(name="p_h", bufs=1) as p_h:
            h1 = [p_h.tile([P, 36 * WP + 2], f32r, tag=f"h1_{j}",
                           name=f"h1_{j}") for j in range(2)]
            h2 = [p_h.tile([P, 34 * WP + 2], f32r, tag=f"h2_{j}",
                           name=f"h2_{j}") for j in range(2)]

            # ---- conv1: 768 -> 256, relu ----
            with tc.tile_pool(name="c1w", bufs=1) as c1w, \
                    tc.tile_pool(name="c1x", bufs=2) as c1x, \
                    tc.tile_pool(name="c1p", bufs=4, space="PSUM") as c1p:
                w1_sb = c1w.tile([P, 6 * 9 * 2, P], f32r)
                nc.gpsimd.dma_start(out=w1_sb[:], in_=w1l[:])

                total = 36 * WP
                p0 = 0
                while p0 < total:
                    size = min(NB, total - p0)
                    rb = p0 // WP          # block starts at a row boundary
                    xts = []
                    for cc in range(6):
                        xt = c1x.tile([P, 5 * WP + 2], f32r, tag=f"xt{cc}",
                                      name=f"xt{cc}")
                        nc.gpsimd.dma_start(
                            out=xt[:],
                            in_=xin[cc, :, rb * WP:rb * WP + 5 * WP + 2])
                        xts.append(xt)

                    for oj in range(2):
                        ps = c1p.tile([P, 512], f32, tag="cpsum", name="ps1")
                        i = 0
                        for cc in range(6):
                            for t in range(9):
                                ty, tx = t // 3, t % 3
                                roff = 1 + ty * WP + (tx - 1)
                                nc.tensor.matmul(
                                    ps[:, :size],
                                    w1_sb[:, (cc * 9 + t) * 2 + oj, :],
                                    xts[cc][:, roff:roff + size],
                                    start=(i == 0), stop=(i == 53))
                                i += 1
                        nc.scalar.activation(
                            h1[oj][:, 1 + p0:1 + p0 + size], ps[:, :size],
                            AF.Relu, bias=b1_sb[:, oj:oj + 1])
                    p0 += size

            # zero h1 outside the image (reference pads h1 with zeros)
            with tc.tile_pool(name="pm1", bufs=1) as pm1:
                m1_sb = pm1.tile([P, 36 * WP], f32r)
                nc.gpsimd.dma_start(out=m1_sb[:], in_=m1m[:])
                for oj in range(2):
                    nc.vector.tensor_tensor(
                        out=h1[oj][:, 1:1 + 36 * WP],
                        in0=h1[oj][:, 1:1 + 36 * WP],
                        in1=m1_sb[:], op=ALU.mult)
                    # pad elements (read by conv2 at never-used positions)
                    # must be finite f32r: copy zeros from the mask edge
                    nc.vector.tensor_copy(out=h1[oj][:, 0:1],
                                          in_=m1_sb[:, 0:1])
                    nc.vector.tensor_copy(out=h1[oj][:, 1 + 36 * WP:],
                                          in_=m1_sb[:, 0:1])

            # ---- conv2: 256 -> 256, relu ----
            with tc.tile_pool(name="c2w", bufs=1) as c2w, \
                    tc.tile_pool(name="c2p", bufs=4, space="PSUM") as c2p:
                w2_sb = c2w.tile([P, 2 * 9 * 2, P], f32r)
                nc.gpsimd.dma_start(out=w2_sb[:], in_=w2l[:])

                def h2_write(oj, p0, size, ps):
                    nc.scalar.activation(
                        h2[oj][:, 1 + p0:1 + p0 + size], ps[:, :size],
                        AF.Relu, bias=b2_sb[:, oj:oj + 1])

                _conv_pass(nc, c2p, h1, w2_sb, 2, 2, 9, 34, h2_write)

            # zero h2 outside the image
            with tc.tile_pool(name="pm2", bufs=1) as pm2:
                m2_sb = pm2.tile([P, 34 * WP], f32r)
                nc.gpsimd.dma_start(out=m2_sb[:], in_=m2m[:])
                for oj in range(2):
                    nc.vector.tensor_tensor(
                        out=h2[oj][:, 1:1 + 34 * WP],
                        in0=h2[oj][:, 1:1 + 34 * WP],
                        in1=m2_sb[:], op=ALU.mult)
                    nc.vector.tensor_copy(out=h2[oj][:, 0:1],
                                          in_=m2_sb[:, 0:1])
                    nc.vector.tensor_copy(out=h2[oj][:, 1 + 34 * WP:],
                                          in_=m2_sb[:, 0:1])

            # ---- conv3: 256 -> 36 (offsets) ----
            with tc.tile_pool(name="c3w", bufs=1) as c3w, \
                    tc.tile_pool(name="c3p", bufs=4, space="PSUM") as c3p:
                w3_sb = c3w.tile([P, 2 * 9 * 1, 50], f32r)
                nc.gpsimd.dma_start(out=w3_sb[:], in_=w3l[:])

                def off_write(oj, p0, size, ps):
                    nc.scalar.activation(
                        off[:, p0:p0 + size], ps[:50, :size],
                        AF.Identity, bias=b3_sb[:, 0:1])

                _conv_pass(nc, c3p, h2, w3_sb, 2, 1, 9, 32, off_write)

        # ============== deformable convs ==============
        p_do = es.enter_context(tc.tile_pool(name="p_do", bufs=1))
        dcnout = [[p_do.tile([P, SR * W], f32r, tag=f"dcn{d}_{oj}",
                             name=f"dcn{d}_{oj}")
                   for oj in range(2)] for d in range(2)]
        offv = off[:].rearrange("p (r w) -> p r w", w=WP)

        for d in range(2):
            with tc.tile_pool(name="dwp", bufs=1) as dwp, \
                    tc.tile_pool(name="dix", bufs=2) as dix, \
                    tc.tile_pool(name="dga", bufs=2) as dga, \
                    tc.tile_pool(name="dst", bufs=2) as dst, \
                    tc.tile_pool(name="dtm", bufs=2) as dtm, \
                    tc.tile_pool(name="dps", bufs=2, space="PSUM") as dps, \
                    tc.tile_pool(name="tps", bufs=2, space="PSUM") as tps:
                wd_sb = dwp.tile([P, 9 * 2 * 2, P], f32r, tag="wd", name="wd")
                nc.gpsimd.dma_start(out=wd_sb[:], in_=wdl[d][:])

                # compact offsets [18, 32, 128], transpose to [128, 32, 18]
                offc = dwp.tile([18, SR, W], f32, tag="offc", name="offc")
                nc.vector.tensor_copy(
                    out=offc[:],
                    in_=offv[32 * d:32 * d + 18, :, XOFF:XOFF + W])
                ot = dwp.tile([P, SR, 18], f32, tag="ot", name="ot")
                for j in range(SR):
                    pt = tps.tile([P, 18], f32, tag="tp", name="ptof")
                    nc.tensor.transpose(pt[:], offc[:, j, :], id_sb[:18, :18])
                    nc.scalar.activation(ot[:, j, :], pt[:], AF.Copy)

                for g in range(8):
                    dyT = ot[:, 4 * g:4 * g + 4, 0:18:2]
                    dxT = ot[:, 4 * g:4 * g + 4, 1:18:2]

                    def tmp(nm):
                        return dtm.tile([P, 36], f32, tag=nm, name=nm)

                    # y side
                    ys = tmp("ys")
                    nc.vector.tensor_tensor(out=ys[:], in0=dyT, in1=ky_sb[:],
                                            op=ALU.add)
                    nc.vector.tensor_scalar(
                        out=ys[:], in0=ys[:], scalar1=mt_sb[:, 1:2],
                        scalar2=float(4 * g), op0=ALU.add, op1=ALU.add)
                    # floor(ys) via round-to-nearest magic: round(ys-0.5).
                    # At integer ys the -1 ambiguity is benign (that corner
                    # gets bilinear weight 0/1 consistently).
                    y0 = tmp("y0")
                    nc.vector.tensor_scalar(out=y0[:], in0=ys[:],
                                            scalar1=-0.5, scalar2=MAGIC,
                                            op0=ALU.add, op1=ALU.add)
                    nc.vector.tensor_scalar(out=y0[:], in0=y0[:],
                                            scalar1=-MAGIC, scalar2=None,
                                            op0=ALU.add)
                    fy = tmp("fy")
                    nc.vector.tensor_tensor(out=fy[:], in0=ys[:], in1=y0[:],
                                            op=ALU.subtract)
                    y0c = tmp("y0c")
                    nc.vector.tensor_scalar(out=y0c[:], in0=y0[:], scalar1=0.0,
                                            scalar2=127.0, op0=ALU.max,
                                            op1=ALU.min)
                    vy0 = tmp("vy0")
                    nc.vector.tensor_tensor(out=vy0[:], in0=y0[:], in1=y0c[:],
                                            op=ALU.is_equal)
                    y1 = tmp("y1")
                    nc.vector.tensor_scalar(out=y1[:], in0=y0[:], scalar1=1.0,
                                            scalar2=None, op0=ALU.add)
                    y1c = tmp("y1c")
                    nc.vector.tensor_scalar(out=y1c[:], in0=y1[:], scalar1=0.0,
                                            scalar2=127.0, op0=ALU.max,
                                            op1=ALU.min)
                    vy1 = tmp("vy1")
                    nc.vector.tensor_tensor(out=vy1[:], in0=y1[:], in1=y1c[:],
                                            op=ALU.is_equal)
                    # x side
                    xs = tmp("xs")
                    nc.vector.tensor_tensor(out=xs[:], in0=dxT, in1=xk_sb[:],
                                            op=ALU.add)
                    x0 = tmp("x0")
                    nc.vector.tensor_scalar(out=x0[:], in0=xs[:],
                                            scalar1=-0.5, scalar2=MAGIC,
                                            op0=ALU.add, op1=ALU.add)
                    nc.vector.tensor_scalar(out=x0[:], in0=x0[:],
                                            scalar1=-MAGIC, scalar2=None,
                                            op0=ALU.add)
                    fx = tmp("fx")
                    nc.vector.tensor_tensor(out=fx[:], in0=xs[:], in1=x0[:],
                                            op=ALU.subtract)
                    x0c = tmp("x0c")
                    nc.vector.tensor_scalar(out=x0c[:], in0=x0[:], scalar1=0.0,
                                            scalar2=127.0, op0=ALU.max,
                                            op1=ALU.min)
                    vx0 = tmp("vx0")
                    nc.vector.tensor_tensor(out=vx0[:], in0=x0[:], in1=x0c[:],
                                            op=ALU.is_equal)
                    x1 = tmp("x1")
                    nc.vector.tensor_scalar(out=x1[:], in0=x0[:], scalar1=1.0,
                                            scalar2=None, op0=ALU.add)
                    x1c = tmp("x1c")
                    nc.vector.tensor_scalar(out=x1c[:], in0=x1[:], scalar1=0.0,
                                            scalar2=127.0, op0=ALU.max,
                                            op1=ALU.min)
                    vx1 = tmp("vx1")
                    nc.vector.tensor_tensor(out=vx1[:], in0=x1[:], in1=x1c[:],
                                            op=ALU.is_equal)

                    # interpolation weights (masked by validity)
                    wy0 = tmp("wy0")
                    nc.vector.tensor_scalar(out=wy0[:], in0=fy[:],
                                            scalar1=-1.0, scalar2=1.0,
                                            op0=ALU.mult, op1=ALU.add)
                    nc.vector.tensor_tensor(out=wy0[:], in0=wy0[:],
                                            in1=vy0[:], op=ALU.mult)
                    wy1 = tmp("wy1")
                    nc.vector.tensor_tensor(out=wy1[:], in0=fy[:], in1=vy1[:],
                                            op=ALU.mult)
                    wx0 = tmp("wx0")
                    nc.vector.tensor_scalar(out=wx0[:], in0=fx[:],
                                            scalar1=-1.0, scalar2=1.0,
                                            op0=ALU.mult, op1=ALU.add)
                    nc.vector.tensor_tensor(out=wx0[:], in0=wx0[:],
                                            in1=vx0[:], op=ALU.mult)
                    wx1 = tmp("wx1")
                    nc.vector.tensor_tensor(out=wx1[:], in0=fx[:], in1=vx1[:],
                                            op=ALU.mult)

                    wgt = dix.tile([P, 144], f32, tag="wgt", name="wgt")
                    wv4 = wgt[:].rearrange("p (x c) -> p x c", c=4)
                    for ci, (wy, wx) in enumerate(
                            [(wy0, wx0), (wy0, wx1), (wy1, wx0), (wy1, wx1)]):
                        nc.vector.tensor_tensor(
                            out=wv4[:, :, ci], in0=wy[:], in1=wx[:],
                            op=ALU.mult)

                    # gather indices: (yc*W + xc) + b*HW, exact in fp32
                    idx = dix.tile([P, 144], i32, tag="idx", name="idx")
                    iv4 = idx[:].rearrange("p (x c) -> p x c", c=4)
                    ifl = tmp("ifl")
                    for ci, (yc, xc) in enumerate(
                            [(y0c, x0c), (y0c, x1c), (y1c, x0c), (y1c, x1c)]):
                        nc.vector.tensor_scalar(out=ifl[:], in0=yc[:],
                                                scalar1=float(W),
                                                scalar2=None, op0=ALU.mult)
                        nc.vector.tensor_tensor(out=ifl[:], in0=ifl[:],
                                                in1=xc[:], op=ALU.add)
                        nc.vector.tensor_scalar(
                            out=iv4[:, :, ci], in0=ifl[:],
                            scalar1=mt_sb[:, 0:1], scalar2=None, op0=ALU.add)

                    pso = [dps.tile([P, 512], f32, tag=f"acc{oj}",
                                    name=f"acc{oj}") for oj in range(2)]
                    for k in range(9):
                        rg = dga.tile([P, 16 * C], f32, tag="rg", name="rg")
                        # HW consumes ONE dynamic offset per partition per
                        # indirect DMA: issue one call per (row j, corner c)
                        # chunk, spread over SWDGE queues.
                        for jj in range(4):
                            for ci in range(4):
                                fo = (jj * 9 + k) * 4 + ci
                                nc.gpsimd.indirect_dma_start(
                                    out=rg[:, (jj * 4 + ci) * C:
                                            (jj * 4 + ci + 1) * C],
                                    out_offset=None,
                                    in_=ftp[d][:, :],
                                    in_offset=IndirectOffsetOnAxis(
                                        ap=idx[:, fo:fo + 1], axis=0),

                                )
                        st = [dst.tile([P, 4 * P], f32r, tag=f"st{cc}",
                                       name=f"st{cc}") for cc in range(2)]
                        for j in range(4):
                            m0 = dtm.tile([P, C], f32, tag="m0", name="m0")
                            nc.scalar.activation(
                                m0[:], rg[:, bass.ts(4 * j + 0, C)], AF.Copy,
                                scale=wgt[:, (j * 9 + k) * 4 + 0:(j * 9 + k) * 4 + 1])
                            m1 = dtm.tile([P, C], f32, tag="m1", name="m1")
                            nc.vector.tensor_scalar(
                                out=m1[:], in0=rg[:, bass.ts(4 * j + 1, C)],
                                scalar1=wgt[:, (j * 9 + k) * 4 + 1:(j * 9 + k) * 4 + 2], scalar2=None,
                                op0=ALU.mult)
                            m2 = dtm.tile([P, C], f32, tag="m2", name="m2")
                            nc.scalar.activation(
                                m2[:], rg[:, bass.ts(4 * j + 2, C)], AF.Copy,
                                scale=wgt[:, (j * 9 + k) * 4 + 2:(j * 9 + k) * 4 + 3])
                            m3 = dtm.tile([P, C], f32, tag="m3", name="m3")
                            nc.vector.tensor_scalar(
                                out=m3[:], in0=rg[:, bass.ts(4 * j + 3, C)],
                                scalar1=wgt[:, (j * 9 + k) * 4 + 3:(j * 9 + k) * 4 + 4], scalar2=None,
                                op0=ALU.mult)
                            nc.vector.tensor_tensor(out=m0[:], in0=m0[:],
                                                    in1=m1[:], op=ALU.add)
                            nc.vector.tensor_tensor(out=m2[:], in0=m2[:],
                                                    in1=m3[:], op=ALU.add)
                            nc.vector.tensor_tensor(out=m0[:], in0=m0[:],
                                                    in1=m2[:], op=ALU.add)
                            for cc in range(2):
                                pt = tps.tile([P, P], f32, tag="tpS",
                                              name="ptS")
                                nc.tensor.transpose(
                                    pt[:], m0[:, cc * P:cc * P + P],
                                    id_sb[:])
                                nc.scalar.activation(
                                    st[cc][:, j * P:j * P + P], pt[:],
                                    AF.Copy)
                        for cc in range(2):
                            for oj in range(2):
                                nc.tensor.matmul(
                                    pso[oj][:],
                                    wd_sb[:, (k * 2 + cc) * 2 + oj, :],
                                    st[cc][:],
                                    start=(k == 0 and cc == 0),
                                    stop=(k == 8 and cc == 1))
                    for oj in range(2):
                        nc.scalar.activation(
                            dcnout[d][oj][:, g * 512:(g + 1) * 512],
                            pso[oj][:], AF.Copy)

        # ============== fuse: 1x1 conv 768 -> 256 ==============
        with tc.tile_pool(name="fw", bufs=1) as fw, \
                tc.tile_pool(name="fo", bufs=3) as fo, \
                tc.tile_pool(name="fp", bufs=4, space="PSUM") as fp:
            wf_sb = fw.tile([P, 6 * 2, P], f32r)
            nc.gpsimd.dma_start(out=wf_sb[:], in_=wfl[:])
            f1_sb = [fw.tile([P, SR * W], f32r, tag=f"f1_{j}", name=f"f1_{j}")
                     for j in range(2)]
            for j in range(2):
                nc.gpsimd.dma_start(out=f1_sb[j][:], in_=ft1s[j, :, :])
            frs = [f1_sb[0], f1_sb[1], dcnout[0][0], dcnout[0][1],
                   dcnout[1][0], dcnout[1][1]]
            for nb in range(8):
                sl = slice(nb * 512, (nb + 1) * 512)
                for oj in range(2):
                    ps = fp.tile([P, 512], f32, tag="fps", name="psf")
                    for cc in range(6):
                        nc.tensor.matmul(
                            ps[:], wf_sb[:, cc * 2 + oj, :],
                            frs[cc][:, sl],
                            start=(cc == 0), stop=(cc == 5))
                    ob = fo.tile([P, 512], f32, tag="ob", name="ob")
                    nc.scalar.activation(ob[:], ps[:], AF.Identity,
                                         bias=bf_sb[:, oj:oj + 1])
                    nc.sync.dma_start(out=out[oj, :, sl], in_=ob[:])

    if split_waits:
        _split_sync_waits(nc)
    return nc


# --------------------------------------------------------------------------
# Host-side input prep / output assembly
# --------------------------------------------------------------------------
def prep_inputs(ft_1, ft_2, ft_3, w1, b1, w2, b2, w3, b3, w_dcn2, w_dcn3,
                w_fuse, b_fuse):
    ft_1, ft_2, ft_3 = (np.asarray(a, np.float32) for a in (ft_1, ft_2, ft_3))
    combined = np.concatenate([ft_1, ft_2, ft_3], axis=1)  # [B, 768, H, W]

    def conv_lhsT(wt, n_cc, n_oj, mm):
        # [Cout, Cin, 3, 3] -> [128, n_cc*9*n_oj, mm]
        wt = np.asarray(wt, np.float32)
        cout, cin = wt.shape[0], wt.shape[1]
        a = wt.reshape(n_oj, mm, n_cc, P, 3, 3)
        a = a.transpose(3, 2, 4, 5, 0, 1)  # [ci, cc, ty, tx, oj, o]
        return np.ascontiguousarray(a.reshape(P, n_cc * 9 * n_oj, mm))

    w1l = conv_lhsT(w1, 6, 2, P)
    w2l = conv_lhsT(w2, 2, 2, P)
    # conv3: pad output channels 36 -> 50 so each dcn's 18 offset channels
    # start at a legal partition offset (0 and 32)
    w3p = np.zeros((50, 768 // 3, 3, 3), np.float32)
    w3a = np.asarray(w3, np.float32)
    w3p[0:18] = w3a[0:18]
    w3p[32:50] = w3a[18:36]
    w3l = conv_lhsT(w3p, 2, 1, 50)

    def dcn_lhsT(wt):
        # [256, 256, 3, 3] -> [128ci, (k, cc, oj), 128o]
        wt = np.asarray(wt, np.float32)
        a = wt.reshape(2, P, 2, P, 9)        # [ojb, o, cc, ci, k]
        a = a.transpose(3, 4, 2, 0, 1)       # [ci, k, cc, oj, o]
        return np.ascontiguousarray(a.reshape(P, 9 * 2 * 2, P))

    wd2l = dcn_lhsT(w_dcn2)
    wd3l = dcn_lhsT(w_dcn3)

    wf = np.asarray(w_fuse, np.float32).reshape(256, 768)
    a = wf.reshape(2, P, 6, P).transpose(3, 2, 0, 1)  # [ci, cc, oj, o]
    wfl = np.ascontiguousarray(a.reshape(P, 6 * 2, P))

    b1p = np.ascontiguousarray(np.asarray(b1, np.float32).reshape(2, P).T)
    b2p = np.ascontiguousarray(np.asarray(b2, np.float32).reshape(2, P).T)
    b3p = np.zeros((50, 1), np.float32)
    b3a = np.asarray(b3, np.float32).reshape(36)
    b3p[0:18, 0] = b3a[0:18]
    b3p[32:50, 0] = b3a[18:36]
    bfp = np.ascontiguousarray(np.asarray(b_fuse, np.float32).reshape(2, P).T)

    jj, kk = np.meshgrid(np.arange(4), np.arange(9), indexing="ij")
    kyrow = (jj + kk // 3 - 1).astype(np.float32).reshape(1, 36)
    kyrow = np.ascontiguousarray(np.broadcast_to(kyrow, (P, 36)))
    xkx = (np.arange(P)[:, None] + (kk % 3 - 1).reshape(1, 36)).astype(
        np.float32)
    xkx = np.ascontiguousarray(xkx)
    ident = np.eye(P, dtype=np.float32)

    ftp2 = np.ascontiguousarray(
        ft_2.transpose(0, 2, 3, 1).reshape(B * HW, C))
    ftp3 = np.ascontiguousarray(
        ft_3.transpose(0, 2, 3, 1).reshape(B * HW, C))

    in_maps = []
    for core in range(NCORES):
        b, s = divmod(core, 4)
        r0 = s * SR
        xin = np.zeros((6, P, 38, WP), np.float32)
        lo, hi = r0 - 3, r0 + SR + 3            # conv1 input rows
        vlo, vhi = max(lo, 0), min(hi, H)
        xin[:, :, vlo - lo:vhi - lo, XOFF:XOFF + W] = (
            combined[b, :, vlo:vhi, :].reshape(6, P, vhi - vlo, W))
        ft1s = np.ascontiguousarray(
            ft_1[b, :, r0:r0 + SR, :].reshape(2, P, SR * W))
        mt = np.zeros((P, 2), np.float32)
        mt[:, 0] = b * HW
        mt[:, 1] = r0

        def bmask(rows, rlo):
            m = np.zeros((rows, WP), np.float32)
            for i in range(rows):
                if 0 <= rlo + i < H:
                    m[i, XOFF:XOFF + W] = 1.0
            return np.ascontiguousarray(
                np.broadcast_to(m.reshape(1, rows * WP), (P, rows * WP)))

        m1 = bmask(36, r0 - 2)
        m2 = bmask(34, r0 - 1)
        xinf = np.zeros((6, P, 38 * WP + 2), np.float32)
        xinf[:, :, 1:1 + 38 * WP] = xin.reshape(6, P, 38 * WP)
        in_maps.append({
            "xin": xinf,
            "ft1s": ft1s, "ftp2": ftp2, "ftp3": ftp3,
            "w1l": w1l, "w2l": w2l, "w3l": w3l,
            "wd2l": wd2l, "wd3l": wd3l, "wfl": wfl,
            "b1": b1p, "b2": b2p, "b3": b3p, "bf": bfp,
            "kyrow": kyrow, "xkx": xkx, "meta": mt, "ident": ident,
            "m1m": m1, "m2m": m2,
        })
    return in_maps


def assemble_output(results):
    full = np.empty((B, C, H, W), np.float32)
    for core in range(NCORES):
        b, s = divmod(core, 4)
        r0 = s * SR
        o = results[core]["out"]            # [2, 128, SR*W]
        for oj in range(2):
            full[b, oj * P:(oj + 1) * P, r0:r0 + SR, :] = o[oj].reshape(
                P, SR, W)
    return full


_CACHED_NC = None


def kernel(**inputs) -> np.ndarray:
    global _CACHED_NC
    in_maps = prep_inputs(**inputs)
    if _CACHED_NC is None:
        _CACHED_NC = build_program()
    res = run_bass_kernel_spmd(_CACHED_NC, in_maps, list(range(NCORES)))
    return assemble_output(res.results)


if __name__ == "__main__":
    import json
    rng = np.random.default_rng(0)
    print("building program (syntax check)...")
    nc = build_program()
    print("instructions:",
          sum(len(bb.instructions) for bb in nc.m.functions[0].blocks))



# revision 2
# speedup vs baseline: 1.3433x; 1.3433x over previous
"""Trainium2 Bass kernel for nn_DeformableFusion.

Pipeline (reference): concat(ft1,ft2,ft3) -> conv3x3(768->256)+relu ->
conv3x3(256->256)+relu -> conv3x3(256->36) = offsets -> two deformable
convs (ft_2, ft_3) -> concat(ft_1, a2, a3) -> conv1x1(768->256).

Sharding: 8 cores = 2 samples x 4 row-strips of 32 rows. Each core gets
zero-padded strip inputs for the conv chain plus full pixel-major copies
of ft_2/ft_3 so the deformable gather needs no halo exchange.

All matmuls run as float32r (full-rate fp32). Convs use the flat
shift-accumulation trick on a width-134 zero-padded grid. The bilinear
gather uses indirect DMA (1KB per pixel: all 256 channels of one source
pixel), combined with per-partition scalar weights, then PE-transposed
back to channel-major for the deformable matmul.
"""
import sys
from contextlib import ExitStack

sys.path.insert(0, "/opt/trn_rl_repo")

import numpy as np

import concourse.bass as bass
import concourse.mybir as mybir
from concourse.bass import IndirectOffsetOnAxis
from concourse.bass_utils import run_bass_kernel_spmd
from concourse.tile import TileContext

f32 = mybir.dt.float32
f32r = mybir.dt.float32r  # TF32-like on HW (rel ~1.6e-3): 4x matmul rate vs fp32
i32 = mybir.dt.int32
AF = mybir.ActivationFunctionType
ALU = mybir.AluOpType

P = 128
B, C, H, W = 2, 256, 128, 128
KK = 9
NCORES = 8
SR = 32          # strip rows per core
WP = 134         # padded grid width (x in [-3, 131))
XOFF = 3         # image x -> padded col offset
NB = 402         # conv matmul moving-block size (3 rows of WP)
HW = H * W
MAGIC = 12582912.0  # 1.5 * 2**23, fp32 round-to-int magic


# --------------------------------------------------------------------------
# Walrus in this toolchain rejects instructions carrying more than ~2 sync
# waits ("Too many sync wait commands" on the Tile tail Drain). Spread the
# global-clock waits one-per-NOP before the drain.
# --------------------------------------------------------------------------
def _patch_tile_drain():
    import re

    import bass_rust
    import concourse.tile as tile_mod

    ScopedClock = bass_rust.ScopedClock
    VectorClock = bass_rust.VectorClock

    def _vc_ticks(vc):
        m = re.search(r"VectorClock\(\[(.*)\]\)", repr(vc))
        body = m.group(1).strip()
        return [int(t) for t in body.split(",")] if body else []

    def _drain_and_barrier(self, tick_clock, wait_clock):
        ticks = _vc_ticks(tick_clock.global_clock)
        for proc, tick in enumerate(ticks):
            if tick <= 0:
                continue
            single = [0] * len(ticks)
            single[proc] = tick
            nop = self.nc.sync.nop(nofuse=True, hint=f"drain_wait_p{proc}")
            wait_clock.add_sem_waits(
                nop.ins, ScopedClock({None: VectorClock(single)})
            )
        drain_inst = self.nc.sync.drain()
        wait_clock.add_sem_waits(
            drain_inst.ins,
            ScopedClock({None: tick_clock.global_clock}),
            ScopedClock({None: tick_clock.global_clock.copy()}),
        )
        self.nc.all_engine_barrier()
        assert self.sems is not None
        popped = self.nc._tile_sem_poison_stack.pop()
        assert popped is self._sem_poison
        self.nc.clear_and_free_semaphores(list(self.sems.allocated().values()))
        self.nc.all_engine_barrier()

    tile_mod.TileContext._drain_and_barrier = _drain_and_barrier


_patch_tile_drain()


def _split_sync_waits(nc, cap=1):
    """Walrus in this toolchain caps sync waits per instruction. Hoist
    excess waits onto same-engine NoOps inserted immediately before the
    overloaded instruction (engines are in-order, so waiting earlier on
    the same engine is always safe in this straight-line program)."""
    n = 0
    for bb in nc.m.functions[0].blocks:
        insts = bb.instructions
        i = 0
        while i < len(insts):
            inst = insts[i]
            si = inst.sync_info
            waits = si.on_wait if si is not None else None
            if waits and len(waits) > cap:
                excess = waits[cap:]
                del waits[cap:]
                for j in range(0, len(excess), cap):
                    nop = mybir.InstNoOp(
                        name=f"I-waitsplit-{n}", ins=[], outs=[],
                        engine=inst.engine,
                        sync_info=mybir.SyncInfo(
                            on_wait=excess[j:j + cap], on_update=[]),
                        bass_nofuse=True,
                    )
                    n += 1
                    insts.insert(i, nop)
                    i += 1
            i += 1
    return n


# --------------------------------------------------------------------------
# Device program
# --------------------------------------------------------------------------
def _r(ap):
    return ap.bitcast(f32r)


def _conv_pass(nc, ppool, in_tiles, w_sb, n_cc, n_oj, taps,
               rows_out, out_write):
    """Shift-accumulation 3x3 conv over the flat padded grid.

    in_tiles: per-cc SBUF tiles holding rows_out+2 rows at flat offset 1.
    w_sb: [P, n_cc*9*n_oj*M] weight tile, M = out partition size.
    out_write(oj, p0, size, psum_ap): consume one accumulated block.
    """
    total = rows_out * WP
    m = w_sb.shape[-1]
    p0 = 0
    while p0 < total:
        size = min(NB, total - p0)
        for oj in range(n_oj):
            ps = ppool.tile([P, 512], f32, tag="cpsum", name="psc")
            nmm = n_cc * taps
            i = 0
            for cc in range(n_cc):
                for t in range(taps):
                    ty, tx = t // 3, t % 3
                    roff = 1 + p0 + ty * WP + (tx - 1)
                    nc.tensor.matmul(
                        ps[:m, :size],
                        w_sb[:, ((cc * taps + t) * n_oj + oj), :],
                        in_tiles[cc][:, roff:roff + size],
                        start=(i == 0),
                        stop=(i == nmm - 1),
                    )
                    i += 1
            out_write(oj, p0, size, ps)
        p0 += size


def build_program(split_waits=True):
    nc = bass.Bass("TRN2", target_bir_lowering=False, debug=False,
                   num_devices=NCORES)

    def din(name, shape, dtype=f32):
        return nc.dram_tensor(name, shape, dtype, kind="ExternalInput").ap()

    # conv-chain input strip: 38 rows x 134 cols, zero-padded, 6 c-chunks
    xin = din("xin", [6, P, 38 * WP + 2])
    ft1s = din("ft1s", [2, P, SR * W])          # fuse input strip
    ftp = [din("ftp2", [B * HW, C]), din("ftp3", [B * HW, C])]
    w1l = din("w1l", [P, 6 * 9 * 2, P])
    w2l = din("w2l", [P, 2 * 9 * 2, P])
    w3l = din("w3l", [P, 2 * 9 * 1, 50])
    wdl = [din("wd2l", [P, 9 * 2 * 2, P]), din("wd3l", [P, 9 * 2 * 2, P])]
    wfl = din("wfl", [P, 6 * 2, P])
    b1 = din("b1", [P, 2])
    b2 = din("b2", [P, 2])
    b3 = din("b3", [50, 1])
    bf = din("bf", [P, 2])
    kyrow = din("kyrow", [P, 36])   # (j,k): j + ky[k]
    xkx = din("xkx", [P, 36])       # (j,k): x(part) + kx[k]
    meta = din("meta", [P, 2])      # col0: b*HW (idx offset), col1: r0
    m1m = din("m1m", [P, 36 * WP])  # h1 image-boundary mask (rows+cols)
    m2m = din("m2m", [P, 34 * WP])  # h2 image-boundary mask
    ident = din("ident", [P, P])
    out = nc.dram_tensor("out", [2, P, SR * W], f32, kind="ExternalOutput").ap()

    with TileContext(nc) as tc, ExitStack() as es:
        cst = es.enter_context(tc.tile_pool(name="cst", bufs=1))
        ky_sb = cst.tile([P, 36], f32)
        nc.sync.dma_start(out=ky_sb[:], in_=kyrow[:])
        xk_sb = cst.tile([P, 36], f32)
        nc.sync.dma_start(out=xk_sb[:], in_=xkx[:])
        mt_sb = cst.tile([P, 2], f32)
        nc.sync.dma_start(out=mt_sb[:], in_=meta[:])
        id_sb = cst.tile([P, P], f32)
        nc.sync.dma_start(out=id_sb[:], in_=ident[:])
        b1_sb = cst.tile([P, 2], f32)
        nc.sync.dma_start(out=b1_sb[:], in_=b1[:])
        b2_sb = cst.tile([P, 2], f32)
        nc.sync.dma_start(out=b2_sb[:], in_=b2[:])
        b3_sb = cst.tile([50, 1], f32)
        nc.sync.dma_start(out=b3_sb[:], in_=b3[:])
        bf_sb = cst.tile([P, 2], f32)
        nc.sync.dma_start(out=bf_sb[:], in_=bf[:])

        p_off = es.enter_context(tc.tile_pool(name="p_off", bufs=1))
        off = p_off.tile([50, 32 * WP], f32)

        # ============== conv chain (h1/h2 live only here) ==============
        with tc.tile_pool(name="p_h", bufs=1) as p_h:
            h1 = [p_h.tile([P, 36 * WP + 2], f32r, tag=f"h1_{j}",
                           name=f"h1_{j}") for j in range(2)]
            h2 = [p_h.tile([P, 34 * WP + 2], f32r, tag=f"h2_{j}",
                           name=f"h2_{j}") for j in range(2)]

            # ---- conv1: 768 -> 256, relu ----
            with tc.tile_pool(name="c1w", bufs=1) as c1w, \
                    tc.tile_pool(name="c1x", bufs=2) as c1x, \
                    tc.tile_pool(name="c1p", bufs=4, space="PSUM") as c1p:
                w1_sb = c1w.tile([P, 6 * 9 * 2, P], f32r)
                nc.gpsimd.dma_start(out=w1_sb[:], in_=w1l[:])

                total = 36 * WP
                p0 = 0
                while p0 < total:
                    size = min(NB, total - p0)
                    rb = p0 // WP          # block starts at a row boundary
                    xts = []
                    for cc in range(6):
                        xt = c1x.tile([P, 5 * WP + 2], f32r, tag=f"xt{cc}",
                                      name=f"xt{cc}")
                        nc.gpsimd.dma_start(
                            out=xt[:],
                            in_=xin[cc, :, rb * WP:rb * WP + 5 * WP + 2])
                        xts.append(xt)

                    for oj in range(2):
                        ps = c1p.tile([P, 512], f32, tag="cpsum", name="ps1")
                        i = 0
                        for cc in range(6):
                            for t in range(9):
                                ty, tx = t // 3, t % 3
                                roff = 1 + ty * WP + (tx - 1)
                                nc.tensor.matmul(
                                    ps[:, :size],
                                    w1_sb[:, (cc * 9 + t) * 2 + oj, :],
                                    xts[cc][:, roff:roff + size],
                                    start=(i == 0), stop=(i == 53))
                                i += 1
                        nc.scalar.activation(
                            h1[oj][:, 1 + p0:1 + p0 + size], ps[:, :size],
                            AF.Relu, bias=b1_sb[:, oj:oj + 1])
                    p0 += size

            # zero h1 outside the image (reference pads h1 with zeros)
            with tc.tile_pool(name="pm1", bufs=1) as pm1:
                m1_sb = pm1.tile([P, 36 * WP], f32r)
                nc.gpsimd.dma_start(out=m1_sb[:], in_=m1m[:])
                for oj in range(2):
                    nc.vector.tensor_tensor(
                        out=h1[oj][:, 1:1 + 36 * WP],
                        in0=h1[oj][:, 1:1 + 36 * WP],
                        in1=m1_sb[:], op=ALU.mult)
                    # pad elements (read by conv2 at never-used positions)
                    # must be finite f32r: copy zeros from the mask edge
                    nc.vector.tensor_copy(out=h1[oj][:, 0:1],
                                          in_=m1_sb[:, 0:1])
                    nc.vector.tensor_copy(out=h1[oj][:, 1 + 36 * WP:],
                                          in_=m1_sb[:, 0:1])

            # ---- conv2: 256 -> 256, relu ----
            with tc.tile_pool(name="c2w", bufs=1) as c2w, \
                    tc.tile_pool(name="c2p", bufs=4, space="PSUM") as c2p:
                w2_sb = c2w.tile([P, 2 * 9 * 2, P], f32r)
                nc.gpsimd.dma_start(out=w2_sb[:], in_=w2l[:])

                def h2_write(oj, p0, size, ps):
                    nc.scalar.activation(
                        h2[oj][:, 1 + p0:1 + p0 + size], ps[:, :size],
                        AF.Relu, bias=b2_sb[:, oj:oj + 1])

                _conv_pass(nc, c2p, h1, w2_sb, 2, 2, 9, 34, h2_write)

            # zero h2 outside the image
            with tc.tile_pool(name="pm2", bufs=1) as pm2:
                m2_sb = pm2.tile([P, 34 * WP], f32r)
                nc.gpsimd.dma_start(out=m2_sb[:], in_=m2m[:])
                for oj in range(2):
                    nc.vector.tensor_tensor(
                        out=h2[oj][:, 1:1 + 34 * WP],
                        in0=h2[oj][:, 1:1 + 34 * WP],
                        in1=m2_sb[:], op=ALU.mult)
                    nc.vector.tensor_copy(out=h2[oj][:, 0:1],
                                          in_=m2_sb[:, 0:1])
                    nc.vector.tensor_copy(out=h2[oj][:, 1 + 34 * WP:],
                                          in_=m2_sb[:, 0:1])

            # ---- conv3: 256 -> 36 (offsets) ----
            with tc.tile_pool(name="c3w", bufs=1) as c3w, \
                    tc.tile_pool(name="c3p", bufs=4, space="PSUM") as c3p:
                w3_sb = c3w.tile([P, 2 * 9 * 1, 50], f32r)
                nc.gpsimd.dma_start(out=w3_sb[:], in_=w3l[:])

                def off_write(oj, p0, size, ps):
                    nc.scalar.activation(
                        off[:, p0:p0 + size], ps[:50, :size],
                        AF.Identity, bias=b3_sb[:, 0:1])

                _conv_pass(nc, c3p, h2, w3_sb, 2, 1, 9, 32, off_write)

        # ============== deformable convs ==============
        p_do = es.enter_context(tc.tile_pool(name="p_do", bufs=1))
        dcnout = [[p_do.tile([P, SR * W], f32r, tag=f"dcn{d}_{oj}",
                             name=f"dcn{d}_{oj}")
                   for oj in range(2)] for d in range(2)]
        offv = off[:].rearrange("p (r w) -> p r w", w=WP)

        for d in range(2):
            with tc.tile_pool(name="dwp", bufs=1) as dwp, \
                    tc.tile_pool(name="dix", bufs=2) as dix, \
                    tc.tile_pool(name="dga", bufs=2) as dga, \
                    tc.tile_pool(name="dst", bufs=2) as dst, \
                    tc.tile_pool(name="dtm", bufs=2) as dtm, \
                    tc.tile_pool(name="dps", bufs=2, space="PSUM") as dps, \
                    tc.tile_pool(name="tps", bufs=2, space="PSUM") as tps:
                wd_sb = dwp.tile([P, 9 * 2 * 2, P], f32r, tag="wd", name="wd")
                nc.gpsimd.dma_start(out=wd_sb[:], in_=wdl[d][:])

                # compact offsets [18, 32, 128], transpose to [128, 32, 18]
                offc = dwp.tile([18, SR, W], f32, tag="offc", name="offc")
                nc.vector.tensor_copy(
                    out=offc[:],
                    in_=offv[32 * d:32 * d + 18, :, XOFF:XOFF + W])
                ot = dwp.tile([P, SR, 18], f32, tag="ot", name="ot")
                for j in range(SR):
                    pt = tps.tile([P, 18], f32, tag="tp", name="ptof")
                    nc.tensor.transpose(pt[:], offc[:, j, :], id_sb[:18, :18])
                    nc.scalar.activation(ot[:, j, :], pt[:], AF.Copy)

                for g in range(8):
                    dyT = ot[:, 4 * g:4 * g + 4, 0:18:2]
                    dxT = ot[:, 4 * g:4 * g + 4, 1:18:2]

                    def tmp(nm):
                        return dtm.tile([P, 36], f32, tag=nm, name=nm)

                    # y side
                    ys = tmp("ys")
                    nc.vector.tensor_tensor(out=ys[:], in0=dyT, in1=ky_sb[:],
                                            op=ALU.add)
                    nc.vector.tensor_scalar(
                        out=ys[:], in0=ys[:], scalar1=mt_sb[:, 1:2],
                        scalar2=float(4 * g), op0=ALU.add, op1=ALU.add)
                    # floor(ys) via round-to-nearest magic: round(ys-0.5).
                    # At integer ys the -1 ambiguity is benign (that corner
                    # gets bilinear weight 0/1 consistently).
                    y0 = tmp("y0")
                    nc.vector.tensor_scalar(out=y0[:], in0=ys[:],
                                            scalar1=-0.5, scalar2=MAGIC,
                                            op0=ALU.add, op1=ALU.add)
                    nc.vector.tensor_scalar(out=y0[:], in0=y0[:],
                                            scalar1=-MAGIC, scalar2=None,
                                            op0=ALU.add)
                    fy = tmp("fy")
                    nc.vector.tensor_tensor(out=fy[:], in0=ys[:], in1=y0[:],
                                            op=ALU.subtract)
                    y0c = tmp("y0c")
                    nc.vector.tensor_scalar(out=y0c[:], in0=y0[:], scalar1=0.0,
                                            scalar2=127.0, op0=ALU.max,
                                            op1=ALU.min)
                    vy0 = tmp("vy0")
                    nc.vector.tensor_tensor(out=vy0[:], in0=y0[:], in1=y0c[:],
                                            op=ALU.is_equal)
                    y1 = tmp("y1")
                    nc.vector.tensor_scalar(out=y1[:], in0=y0[:], scalar1=1.0,
                                            scalar2=None, op0=ALU.add)
                    y1c = tmp("y1c")
                    nc.vector.tensor_scalar(out=y1c[:], in0=y1[:], scalar1=0.0,
                                            scalar2=127.0, op0=ALU.max,
                                            op1=ALU.min)
                    vy1 = tmp("vy1")
                    nc.vector.tensor_tensor(out=vy1[:], in0=y1[:], in1=y1c[:],
                                            op=ALU.is_equal)
                    # x side
                    xs = tmp("xs")
                    nc.vector.tensor_tensor(out=xs[:], in0=dxT, in1=xk_sb[:],
                                            op=ALU.add)
                    x0 = tmp("x0")
                    nc.vector.tensor_scalar(out=x0[:], in0=xs[:],
                                            scalar1=-0.5, scalar2=MAGIC,
                                            op0=ALU.add, op1=ALU.add)
                    nc.vector.tensor_scalar(out=x0[:], in0=x0[:],
                                            scalar1=-MAGIC, scalar2=None,
                                            op0=ALU.add)
                    fx = tmp("fx")
                    nc.vector.tensor_tensor(out=fx[:], in0=xs[:], in1=x0[:],
                                            op=ALU.subtract)
                    x0c = tmp("x0c")
                    nc.vector.tensor_scalar(out=x0c[:], in0=x0[:], scalar1=0.0,
                                            scalar2=127.0, op0=ALU.max,
                                            op1=ALU.min)
                    vx0 = tmp("vx0")
                    nc.vector.tensor_tensor(out=vx0[:], in0=x0[:], in1=x0c[:],
                                            op=ALU.is_equal)
                    x1 = tmp("x1")
                    nc.vector.tensor_scalar(out=x1[:], in0=x0[:], scalar1=1.0,
                                            scalar2=None, op0=ALU.add)
                    x1c = tmp("x1c")
                    nc.vector.tensor_scalar(out=x1c[:], in0=x1[:], scalar1=0.0,
                                            scalar2=127.0, op0=ALU.max,
                                            op1=ALU.min)
                    vx1 = tmp("vx1")
                    nc.vector.tensor_tensor(out=vx1[:], in0=x1[:], in1=x1c[:],
                                            op=ALU.is_equal)

                    # interpolation weights (masked by validity)
                    wy0 = tmp("wy0")
                    nc.vector.tensor_scalar(out=wy0[:], in0=fy[:],
                                            scalar1=-1.0, scalar2=1.0,
                                            op0=ALU.mult, op1=ALU.add)
                    nc.vector.tensor_tensor(out=wy0[:], in0=wy0[:],
                                            in1=vy0[:], op=ALU.mult)
                    wy1 = tmp("wy1")
                    nc.vector.tensor_tensor(out=wy1[:], in0=fy[:], in1=vy1[:],
                                            op=ALU.mult)
                    wx0 = tmp("wx0")
                    nc.vector.tensor_scalar(out=wx0[:], in0=fx[:],
                                            scalar1=-1.0, scalar2=1.0,
                                            op0=ALU.mult, op1=ALU.add)
                    nc.vector.tensor_tensor(out=wx0[:], in0=wx0[:],
                                            in1=vx0[:], op=ALU.mult)
                    wx1 = tmp("wx1")
                    nc.vector.tensor_tensor(out=wx1[:], in0=fx[:], in1=vx1[:],
                                            op=ALU.mult)

                    wgt = dix.tile([P, 144], f32, tag="wgt", name="wgt")
                    wv4 = wgt[:].rearrange("p (x c) -> p x c", c=4)
                    for ci, (wy, wx) in enumerate(
                            [(wy0, wx0), (wy0, wx1), (wy1, wx0), (wy1, wx1)]):
                        nc.vector.tensor_tensor(
                            out=wv4[:, :, ci], in0=wy[:], in1=wx[:],
                            op=ALU.mult)

                    # gather indices: (yc*W + xc) + b*HW, exact in fp32
                    idx = dix.tile([P, 144], i32, tag="idx", name="idx")
                    iv4 = idx[:].rearrange("p (x c) -> p x c", c=4)
                    ifl = tmp("ifl")
                    for ci, (yc, xc) in enumerate(
                            [(y0c, x0c), (y0c, x1c), (y1c, x0c), (y1c, x1c)]):
                        nc.vector.tensor_scalar(out=ifl[:], in0=yc[:],
                                                scalar1=float(W),
                                                scalar2=None, op0=ALU.mult)
                        nc.vector.tensor_tensor(out=ifl[:], in0=ifl[:],
                                                in1=xc[:], op=ALU.add)
                        nc.vector.tensor_scalar(
                            out=iv4[:, :, ci], in0=ifl[:],
                            scalar1=mt_sb[:, 0:1], scalar2=None, op0=ALU.add)

                    pso = [dps.tile([P, 512], f32, tag=f"acc{oj}",
                                    name=f"acc{oj}") for oj in range(2)]
                    for k in range(9):
                        rg = dga.tile([P, 16 * C], f32, tag="rg", name="rg")
                        # HW consumes ONE dynamic offset per partition per
                        # indirect DMA: issue one call per (row j, corner c)
                        # chunk, spread over SWDGE queues.
                        for jj in range(4):
                            for ci in range(4):
                                fo = (jj * 9 + k) * 4 + ci
                                nc.gpsimd.indirect_dma_start(
                                    out=rg[:, (jj * 4 + ci) * C:
                                            (jj * 4 + ci + 1) * C],
                                    out_offset=None,
                                    in_=ftp[d][:, :],
                                    in_offset=IndirectOffsetOnAxis(
                                        ap=idx[:, fo:fo + 1], axis=0),

                                )
                        st = [dst.tile([P, 4 * P], f32r, tag=f"st{cc}",
                                       name=f"st{cc}") for cc in range(2)]
                        for j in range(4):
                            m0 = dtm.tile([P, C], f32, tag="m0", name="m0")
                            nc.scalar.activation(
                                m0[:], rg[:, bass.ts(4 * j + 0, C)], AF.Copy,
                                scale=wgt[:, (j * 9 + k) * 4 + 0:(j * 9 + k) * 4 + 1])
                            m1 = dtm.tile([P, C], f32, tag="m1", name="m1")
                            nc.vector.tensor_scalar(
                                out=m1[:], in0=rg[:, bass.ts(4 * j + 1, C)],
                                scalar1=wgt[:, (j * 9 + k) * 4 + 1:(j * 9 + k) * 4 + 2], scalar2=None,
                                op0=ALU.mult)
                            m2 = dtm.tile([P, C], f32, tag="m2", name="m2")
                            nc.scalar.activation(
                                m2[:], rg[:, bass.ts(4 * j + 2, C)], AF.Copy,
                                scale=wgt[:, (j * 9 + k) * 4 + 2:(j * 9 + k) * 4 + 3])
                            m3 = dtm.tile([P, C], f32, tag="m3", name="m3")
                            nc.vector.tensor_scalar(
                                out=m3[:], in0=rg[:, bass.ts(4 * j + 3, C)],
                                scalar1=wgt[:, (j * 9 + k) * 4 + 3:(j * 9 + k) * 4 + 4], scalar2=None,
                                op0=ALU.mult)
                            nc.vector.tensor_tensor(out=m0[:], in0=m0[:],
                                                    in1=m1[:], op=ALU.add)
                            nc.vector.tensor_tensor(out=m2[:], in0=m2[:],
                                                    in1=m3[:], op=ALU.add)
                            nc.vector.tensor_tensor(out=m0[:], in0=m0[:],
                                                    in1=m2[:], op=ALU.add)
                            for cc in range(2):
                                pt = tps.tile([P, P], f32, tag="tpS",
                                              name="ptS")
                                nc.tensor.transpose(
                                    pt[:], m0[:, cc * P:cc * P + P],
                                    id_sb[:])
                                nc.scalar.activation(
                                    st[cc][:, j * P:j * P + P], pt[:],
                                    AF.Copy)
                        for cc in range(2):
                            for oj in range(2):
                                nc.tensor.matmul(
                                    pso[oj][:],
                                    wd_sb[:, (k * 2 + cc) * 2 + oj, :],
                                    st[cc][:],
                                    start=(k == 0 and cc == 0),
                                    stop=(k == 8 and cc == 1))
                    for oj in range(2):
                        nc.scalar.activation(
                            dcnout[d][oj][:, g * 512:(g + 1) * 512],
                            pso[oj][:], AF.Copy)

        # ============== fuse: 1x1 conv 768 -> 256 ==============
        with tc.tile_pool(name="fw", bufs=1) as fw, \
                tc.tile_pool(name="fo", bufs=3) as fo, \
                tc.tile_pool(name="fp", bufs=4, space="PSUM") as fp:
            wf_sb = fw.tile([P, 6 * 2, P], f32r)
            nc.gpsimd.dma_start(out=wf_sb[:], in_=wfl[:])
            f1_sb = [fw.tile([P, SR * W], f32r, tag=f"f1_{j}", name=f"f1_{j}")
                     for j in range(2)]
            for j in range(2):
                nc.gpsimd.dma_start(out=f1_sb[j][:], in_=ft1s[j, :, :])
            frs = [f1_sb[0], f1_sb[1], dcnout[0][0], dcnout[0][1],
                   dcnout[1][0], dcnout[1][1]]
            for nb in range(8):
                sl = slice(nb * 512, (nb + 1) * 512)
                for oj in range(2):
                    ps = fp.tile([P, 512], f32, tag="fps", name="psf")
                    for cc in range(6):
                        nc.tensor.matmul(
                            ps[:], wf_sb[:, cc * 2 + oj, :],
                            frs[cc][:, sl],
                            start=(cc == 0), stop=(cc == 5))
                    ob = fo.tile([P, 512], f32, tag="ob", name="ob")
                    nc.scalar.activation(ob[:], ps[:], AF.Identity,
                                         bias=bf_sb[:, oj:oj + 1])
                    nc.sync.dma_start(out=out[oj, :, sl], in_=ob[:])

    if split_waits:
        _split_sync_waits(nc)
    return nc


# --------------------------------------------------------------------------
# Host-side input prep / output assembly
# --------------------------------------------------------------------------
def prep_inputs(ft_1, ft_2, ft_3, w1, b1, w2, b2, w3, b3, w_dcn2, w_dcn3,
                w_fuse, b_fuse):
    ft_1, ft_2, ft_3 = (np.asarray(a, np.float32) for a in (ft_1, ft_2, ft_3))
    combined = np.concatenate([ft_1, ft_2, ft_3], axis=1)  # [B, 768, H, W]

    def conv_lhsT(wt, n_cc, n_oj, mm):
        # [Cout, Cin, 3, 3] -> [128, n_cc*9*n_oj, mm]
        wt = np.asarray(wt, np.float32)
        cout, cin = wt.shape[0], wt.shape[1]
        a = wt.reshape(n_oj, mm, n_cc, P, 3, 3)
        a = a.transpose(3, 2, 4, 5, 0, 1)  # [ci, cc, ty, tx, oj, o]
        return np.ascontiguousarray(a.reshape(P, n_cc * 9 * n_oj, mm))

    w1l = conv_lhsT(w1, 6, 2, P)
    w2l = conv_lhsT(w2, 2, 2, P)
    # conv3: pad output channels 36 -> 50 so each dcn's 18 offset channels
    # start at a legal partition offset (0 and 32)
    w3p = np.zeros((50, 768 // 3, 3, 3), np.float32)
    w3a = np.asarray(w3, np.float32)
    w3p[0:18] = w3a[0:18]
    w3p[32:50] = w3a[18:36]
    w3l = conv_lhsT(w3p, 2, 1, 50)

    def dcn_lhsT(wt):
        # [256, 256, 3, 3] -> [128ci, (k, cc, oj), 128o]
        wt = np.asarray(wt, np.float32)
        a = wt.reshape(2, P, 2, P, 9)        # [ojb, o, cc, ci, k]
        a = a.transpose(3, 4, 2, 0, 1)       # [ci, k, cc, oj, o]
        return np.ascontiguousarray(a.reshape(P, 9 * 2 * 2, P))

    wd2l = dcn_lhsT(w_dcn2)
    wd3l = dcn_lhsT(w_dcn3)

    wf = np.asarray(w_fuse, np.float32).reshape(256, 768)
    a = wf.reshape(2, P, 6, P).transpose(3, 2, 0, 1)  # [ci, cc, oj, o]
    wfl = np.ascontiguousarray(a.reshape(P, 6 * 2, P))

    b1p = np.ascontiguousarray(np.asarray(b1, np.float32).reshape(2, P).T)
    b2p = np.ascontiguousarray(np.asarray(b2, np.float32).reshape(2, P).T)
    b3p = np.zeros((50, 1), np.float32)
    b3a = np.asarray(b3, np.float32).reshape(36)
    b3p[0:18, 0] = b3a[0:18]
    b3p[32:50, 0] = b3a[18:36]
    bfp = np.ascontiguousarray(np.asarray(b_fuse, np.float32).reshape(2, P).T)

    jj, kk = np.meshgrid(np.arange(4), np.arange(9), indexing="ij")
    kyrow = (jj + kk // 3 - 1).astype(np.float32).reshape(1, 36)
    kyrow = np.ascontiguousarray(np.broadcast_to(kyrow, (P, 36)))
    xkx = (np.arange(P)[:, None] + (kk % 3 - 1).reshape(1, 36)).astype(
        np.float32)
    xkx = np.ascontiguousarray(xkx)
    ident = np.eye(P, dtype=np.float32)

    ftp2 = np.ascontiguousarray(
        ft_2.transpose(0, 2, 3, 1).reshape(B * HW, C))
    ftp3 = np.ascontiguousarray(
        ft_3.transpose(0, 2, 3, 1).reshape(B * HW, C))

    in_maps = []
    for core in range(NCORES):
        b, s = divmod(core, 4)
        r0 = s * SR
        xin = np.zeros((6, P, 38, WP), np.float32)
        lo, hi = r0 - 3, r0 + SR + 3            # conv1 input rows
        vlo, vhi = max(lo, 0), min(hi, H)
        xin[:, :, vlo - lo:vhi - lo, XOFF:XOFF + W] = (
            combined[b, :, vlo:vhi, :].reshape(6, P, vhi - vlo, W))
        ft1s = np.ascontiguousarray(
            ft_1[b, :, r0:r0 + SR, :].reshape(2, P, SR * W))
        mt = np.zeros((P, 2), np.float32)
        mt[:, 0] = b * HW
        mt[:, 1] = r0

        def bmask(rows, rlo):
            m = np.zeros((rows, WP), np.float32)
            for i in range(rows):
                if 0 <= rlo + i < H:
                    m[i, XOFF:XOFF + W] = 1.0
            return np.ascontiguousarray(
                np.broadcast_to(m.reshape(1, rows * WP), (P, rows * WP)))

        m1 = bmask(36, r0 - 2)
        m2 = bmask(34, r0 - 1)
        xinf = np.zeros((6, P, 38 * WP + 2), np.float32)
        xinf[:, :, 1:1 + 38 * WP] = xin.reshape(6, P, 38 * WP)
        in_maps.append({
            "xin": xinf,
            "ft1s": ft1s, "ftp2": ftp2, "ftp3": ftp3,
            "w1l": w1l, "w2l": w2l, "w3l": w3l,
            "wd2l": wd2l, "wd3l": wd3l, "wfl": wfl,
            "b1": b1p, "b2": b2p, "b3": b3p, "bf": bfp,
            "kyrow": kyrow, "xkx": xkx, "meta": mt, "ident": ident,
            "m1m": m1, "m2m": m2,
        })
    return in_maps


def assemble_output(results):
    full = np.empty((B, C, H, W), np.float32)
    for core in range(NCORES):
        b, s = divmod(core, 4)
        r0 = s * SR
        o = results[core]["out"]            # [2, 128, SR*W]
        for oj in range(2):
            full[b, oj * P:(oj + 1) * P, r0:r0 + SR, :] = o[oj].reshape(
                P, SR, W)
    return full


_CACHED_NC = None


def kernel(**inputs) -> np.ndarray:
    global _CACHED_NC
    in_maps = prep_inputs(**inputs)
    if _CACHED_NC is None:
        _CACHED_NC = build_program()
    res = run_bass_kernel_spmd(_CACHED_NC, in_maps, list(range(NCORES)))
    return assemble_output(res.results)


if __name__ == "__main__":
    import json
    rng = np.random.default_rng(0)
    print("building program (syntax check)...")
    nc = build_program()
    print("instructions:",
          sum(len(bb.instructions) for bb in nc.m.functions[0].blocks))



# revision 32
# speedup vs baseline: 1.3670x; 1.0176x over previous
"""Trainium2 Bass kernel for nn_DeformableFusion.

Pipeline (reference): concat(ft1,ft2,ft3) -> conv3x3(768->256)+relu ->
conv3x3(256->256)+relu -> conv3x3(256->36) = offsets -> two deformable
convs (ft_2, ft_3) -> concat(ft_1, a2, a3) -> conv1x1(768->256).

Sharding: 8 cores = 2 samples x 4 row-strips of 32 rows. Each core gets
zero-padded strip inputs for the conv chain plus full PADDED pixel-major
bf16 copies of ft_2/ft_3 (130x130 grid, zero border) so the deformable
gather needs no halo exchange and no validity masks: clamped corner
coords land on zero pixels, so out-of-image corners contribute 0
automatically.

Conv chain runs in float32r (full-rate ~TF32). The deformable path runs
in bf16: one batched dma_gather per (4-row group, tap) fetches 1024
x-adjacent pixel PAIRS (1KB descriptors) using int16 indices packed into
the SWDGE 16-partition wrap layout via a small DRAM bounce.
"""
import sys
from contextlib import ExitStack

sys.path.insert(0, "/opt/trn_rl_repo")

import numpy as np

import concourse.bass as bass
import concourse.mybir as mybir
from concourse.bass_utils import run_bass_kernel_spmd
from concourse.tile import TileContext

f32 = mybir.dt.float32
f32r = mybir.dt.float32r  # TF32-like on HW (rel ~1.6e-3): 4x matmul rate vs fp32
bf16 = mybir.dt.bfloat16
i16 = mybir.dt.int16
AF = mybir.ActivationFunctionType
ALU = mybir.AluOpType

P = 128
B, C, H, W = 2, 256, 128, 128
KK = 9
NCORES = 8
SR = 32          # strip rows per core
WP = 134         # padded grid width (x in [-3, 131))
XOFF = 3         # image x -> padded col offset
NB = 402         # conv matmul moving-block size (3 rows of WP)
HW = H * W
GP = 130         # padded gather grid (y', x' in [0, 130); zero border)
GR = GP * GP     # 16900 rows per (batch, tensor)
MAGIC = 12582912.0  # 1.5 * 2**23, fp32 round-to-int magic


# --------------------------------------------------------------------------
# Walrus in this toolchain rejects instructions carrying more than ~2 sync
# waits ("Too many sync wait commands" on the Tile tail Drain). Spread the
# global-clock waits one-per-NOP before the drain.
# --------------------------------------------------------------------------
def _patch_tile_drain():
    import re

    import bass_rust
    import concourse.tile as tile_mod

    ScopedClock = bass_rust.ScopedClock
    VectorClock = bass_rust.VectorClock

    def _vc_ticks(vc):
        m = re.search(r"VectorClock\(\[(.*)\]\)", repr(vc))
        body = m.group(1).strip()
        return [int(t) for t in body.split(",")] if body else []

    def _drain_and_barrier(self, tick_clock, wait_clock):
        ticks = _vc_ticks(tick_clock.global_clock)
        for proc, tick in enumerate(ticks):
            if tick <= 0:
                continue
            single = [0] * len(ticks)
            single[proc] = tick
            nop = self.nc.sync.nop(nofuse=True, hint=f"drain_wait_p{proc}")
            wait_clock.add_sem_waits(
                nop.ins, ScopedClock({None: VectorClock(single)})
            )
        drain_inst = self.nc.sync.drain()
        wait_clock.add_sem_waits(
            drain_inst.ins,
            ScopedClock({None: tick_clock.global_clock}),
            ScopedClock({None: tick_clock.global_clock.copy()}),
        )
        self.nc.all_engine_barrier()
        assert self.sems is not None
        popped = self.nc._tile_sem_poison_stack.pop()
        assert popped is self._sem_poison
        self.nc.clear_and_free_semaphores(list(self.sems.allocated().values()))
        self.nc.all_engine_barrier()

    tile_mod.TileContext._drain_and_barrier = _drain_and_barrier


_patch_tile_drain()


def _split_sync_waits(nc, cap=1):
    """Walrus in this toolchain caps sync waits per instruction. Hoist
    excess waits onto same-engine NoOps inserted immediately before the
    overloaded instruction (engines are in-order, so waiting earlier on
    the same engine is always safe in this straight-line program)."""
    n = 0
    for bb in nc.m.functions[0].blocks:
        insts = bb.instructions
        i = 0
        while i < len(insts):
            inst = insts[i]
            si = inst.sync_info
            waits = si.on_wait if si is not None else None
            if waits and len(waits) > cap:
                excess = waits[cap:]
                del waits[cap:]
                for j in range(0, len(excess), cap):
                    nop = mybir.InstNoOp(
                        name=f"I-waitsplit-{n}", ins=[], outs=[],
                        engine=inst.engine,
                        sync_info=mybir.SyncInfo(
                            on_wait=excess[j:j + cap], on_update=[]),
                        bass_nofuse=True,
                    )
                    n += 1
                    insts.insert(i, nop)
                    i += 1
            i += 1
    return n


# --------------------------------------------------------------------------
# Device program
# --------------------------------------------------------------------------
def _conv_pass(nc, ppool, in_tiles, w_sb, n_cc, n_oj, taps,
               rows_out, out_write):
    """Shift-accumulation 3x3 conv over the flat padded grid."""
    total = rows_out * WP
    m = w_sb.shape[-1]
    p0 = 0
    while p0 < total:
        size = min(NB, total - p0)
        for oj in range(n_oj):
            ps = ppool.tile([P, 512], f32, tag="cpsum", name="psc")
            nmm = n_cc * taps
            i = 0
            for cc in range(n_cc):
                for t in range(taps):
                    ty, tx = t // 3, t % 3
                    roff = 1 + p0 + ty * WP + (tx - 1)
                    nc.tensor.matmul(
                        ps[:m, :size],
                        w_sb[:, ((cc * taps + t) * n_oj + oj), :],
                        in_tiles[cc][:, roff:roff + size],
                        start=(i == 0),
                        stop=(i == nmm - 1),
                    )
                    i += 1
            out_write(oj, p0, size, ps)
        p0 += size


def build_program(split_waits=True):
    nc = bass.Bass("TRN2", target_bir_lowering=False, debug=False,
                   num_devices=NCORES)

    def din(name, shape, dtype=f32):
        return nc.dram_tensor(name, shape, dtype, kind="ExternalInput").ap()

    # conv-chain input strip: 38 rows x 134 cols, zero-padded, 6 c-chunks
    xin = din("xin", [6, P, 38 * WP + 2])
    ft1s = din("ft1s", [2, P, SR * W], bf16)    # fuse input strip (bf16)
    fpp = [din("fp2p", [GR - 1, 2 * C], bf16),
           din("fp3p", [GR - 1, 2 * C], bf16)]
    w1l = din("w1l", [P, 6 * 9 * 2, P])
    w2l = din("w2l", [P, 2 * 9 * 2, P])
    w3l = din("w3l", [P, 2 * 9 * 1, 50])
    wdl = [din("wd2l", [P, 9 * 2 * 2, P], bf16),
           din("wd3l", [P, 9 * 2 * 2, P], bf16)]
    wfl = din("wfl", [P, 6 * 2, P], bf16)
    b1 = din("b1", [P, 2])
    b2 = din("b2", [P, 2])
    b3 = din("b3", [50, 1])
    bf = din("bf", [P, 2])
    kyrow = din("kyrow", [P, 36])   # (j,k): j + ky[k]
    xkx = din("xkx", [P, 36])       # (j,k): x(part) + kx[k]
    meta = din("meta", [P, 2])      # col1: r0 (col0 unused)
    m1m = din("m1m", [P, 36 * WP])  # h1 image-boundary mask (rows+cols)
    m2m = din("m2m", [P, 34 * WP])  # h2 image-boundary mask
    ident = din("ident", [P, P])
    identb = din("identb", [P, P], bf16)
    out = nc.dram_tensor("out", [2, P, SR * W], f32, kind="ExternalOutput").ap()

    with TileContext(nc) as tc, ExitStack() as es:
        cst = es.enter_context(tc.tile_pool(name="cst", bufs=1))
        ky_sb = cst.tile([P, 36], f32)
        nc.sync.dma_start(out=ky_sb[:], in_=kyrow[:])
        xk_sb = cst.tile([P, 36], f32)
        nc.sync.dma_start(out=xk_sb[:], in_=xkx[:])
        mt_sb = cst.tile([P, 2], f32)
        nc.sync.dma_start(out=mt_sb[:], in_=meta[:])
        id_sb = cst.tile([P, P], f32)
        nc.sync.dma_start(out=id_sb[:], in_=ident[:])
        idb_sb = cst.tile([P, P], bf16)
        nc.sync.dma_start(out=idb_sb[:], in_=identb[:])
        b1_sb = cst.tile([P, 2], f32)
        nc.sync.dma_start(out=b1_sb[:], in_=b1[:])
        b2_sb = cst.tile([P, 2], f32)
        nc.sync.dma_start(out=b2_sb[:], in_=b2[:])
        b3_sb = cst.tile([50, 1], f32)
        nc.sync.dma_start(out=b3_sb[:], in_=b3[:])
        bf_sb = cst.tile([P, 2], f32)
        nc.sync.dma_start(out=bf_sb[:], in_=bf[:])

        p_off = es.enter_context(tc.tile_pool(name="p_off", bufs=1))
        off = p_off.tile([50, 32 * WP], f32)

        # ============== conv chain (h1/h2 live only here) ==============
        with tc.tile_pool(name="p_h", bufs=1) as p_h:
            h1 = [p_h.tile([P, 36 * WP + 2], f32r, tag=f"h1_{j}",
                           name=f"h1_{j}") for j in range(2)]
            h2 = [p_h.tile([P, 34 * WP + 2], f32r, tag=f"h2_{j}",
                           name=f"h2_{j}") for j in range(2)]

            # ---- conv1: 768 -> 256, relu ----
            with tc.tile_pool(name="c1w", bufs=1) as c1w, \
                    tc.tile_pool(name="c1x", bufs=2) as c1x, \
                    tc.tile_pool(name="c1p", bufs=4, space="PSUM") as c1p:
                w1_sb = c1w.tile([P, 6 * 9 * 2, P], f32r)
                nc.gpsimd.dma_start(out=w1_sb[:], in_=w1l[:])

                total = 36 * WP
                p0 = 0
                while p0 < total:
                    size = min(NB, total - p0)
                    rb = p0 // WP          # block starts at a row boundary
                    xts = []
                    for cc in range(6):
                        xt = c1x.tile([P, 5 * WP + 2], f32r, tag=f"xt{cc}",
                                      name=f"xt{cc}")
                        nc.gpsimd.dma_start(
                            out=xt[:],
                            in_=xin[cc, :, rb * WP:rb * WP + 5 * WP + 2])
                        xts.append(xt)

                    for oj in range(2):
                        ps = c1p.tile([P, 512], f32, tag="cpsum", name="ps1")
                        i = 0
                        for cc in range(6):
                            for t in range(9):
                                ty, tx = t // 3, t % 3
                                roff = 1 + ty * WP + (tx - 1)
                                nc.tensor.matmul(
                                    ps[:, :size],
                                    w1_sb[:, (cc * 9 + t) * 2 + oj, :],
                                    xts[cc][:, roff:roff + size],
                                    start=(i == 0), stop=(i == 53))
                                i += 1
                        nc.scalar.activation(
                            h1[oj][:, 1 + p0:1 + p0 + size], ps[:, :size],
                            AF.Relu, bias=b1_sb[:, oj:oj + 1])
                    p0 += size

            # zero h1 outside the image (reference pads h1 with zeros)
            with tc.tile_pool(name="pm1", bufs=1) as pm1:
                m1_sb = pm1.tile([P, 36 * WP], f32r)
                nc.gpsimd.dma_start(out=m1_sb[:], in_=m1m[:])
                for oj in range(2):
                    nc.vector.tensor_tensor(
                        out=h1[oj][:, 1:1 + 36 * WP],
                        in0=h1[oj][:, 1:1 + 36 * WP],
                        in1=m1_sb[:], op=ALU.mult)
                    nc.vector.tensor_copy(out=h1[oj][:, 0:1],
                                          in_=m1_sb[:, 0:1])
                    nc.vector.tensor_copy(out=h1[oj][:, 1 + 36 * WP:],
                                          in_=m1_sb[:, 0:1])

            # ---- conv2: 256 -> 256, relu ----
            with tc.tile_pool(name="c2w", bufs=1) as c2w, \
                    tc.tile_pool(name="c2p", bufs=4, space="PSUM") as c2p:
                w2_sb = c2w.tile([P, 2 * 9 * 2, P], f32r)
                nc.gpsimd.dma_start(out=w2_sb[:], in_=w2l[:])

                def h2_write(oj, p0, size, ps):
                    nc.scalar.activation(
                        h2[oj][:, 1 + p0:1 + p0 + size], ps[:, :size],
                        AF.Relu, bias=b2_sb[:, oj:oj + 1])

                _conv_pass(nc, c2p, h1, w2_sb, 2, 2, 9, 34, h2_write)

            # zero h2 outside the image
            with tc.tile_pool(name="pm2", bufs=1) as pm2:
                m2_sb = pm2.tile([P, 34 * WP], f32r)
                nc.gpsimd.dma_start(out=m2_sb[:], in_=m2m[:])
                for oj in range(2):
                    nc.vector.tensor_tensor(
                        out=h2[oj][:, 1:1 + 34 * WP],
                        in0=h2[oj][:, 1:1 + 34 * WP],
                        in1=m2_sb[:], op=ALU.mult)
                    nc.vector.tensor_copy(out=h2[oj][:, 0:1],
                                          in_=m2_sb[:, 0:1])
                    nc.vector.tensor_copy(out=h2[oj][:, 1 + 34 * WP:],
                                          in_=m2_sb[:, 0:1])

            # ---- conv3: 256 -> 36 (offsets) ----
            with tc.tile_pool(name="c3w", bufs=1) as c3w, \
                    tc.tile_pool(name="c3p", bufs=4, space="PSUM") as c3p:
                w3_sb = c3w.tile([P, 2 * 9 * 1, 50], f32r)
                nc.gpsimd.dma_start(out=w3_sb[:], in_=w3l[:])

                def off_write(oj, p0, size, ps):
                    nc.scalar.activation(
                        off[:, p0:p0 + size], ps[:50, :size],
                        AF.Identity, bias=b3_sb[:, 0:1])

                _conv_pass(nc, c3p, h2, w3_sb, 2, 1, 9, 32, off_write)

        # ============== deformable convs ==============
        p_do = es.enter_context(tc.tile_pool(name="p_do", bufs=1))
        dcnout = [[p_do.tile([P, SR * W], bf16, tag=f"dcn{d}_{oj}",
                             name=f"dcn{d}_{oj}")
                   for oj in range(2)] for d in range(2)]
        offv = off[:].rearrange("p (r w) -> p r w", w=WP)



        for d in range(2):
            with tc.tile_pool(name="dwp", bufs=1) as dwp, \
                    tc.tile_pool(name="dix", bufs=2) as dix, \
                    tc.tile_pool(name="dga", bufs=2) as dga, \
                    tc.tile_pool(name="dst", bufs=2) as dst, \
                    tc.tile_pool(name="dtm", bufs=3) as dtm, \
                    tc.tile_pool(name="dps", bufs=2, space="PSUM") as dps, \
                    tc.tile_pool(name="tps", bufs=2, space="PSUM") as tps:
                wd_sb = dwp.tile([P, 9 * 2 * 2, P], bf16, tag="wd", name="wd")
                nc.gpsimd.dma_start(out=wd_sb[:], in_=wdl[d][:])

                # compact offsets [18, 32, 128], transpose to [128, 32, 18]
                offc = dwp.tile([18, SR, W], f32, tag="offc", name="offc")
                nc.vector.tensor_copy(
                    out=offc[:],
                    in_=offv[32 * d:32 * d + 18, :, XOFF:XOFF + W])
                ot = dwp.tile([P, SR, 18], f32, tag="ot", name="ot")
                with tc.tile_pool(name="otp", bufs=2, space="PSUM") as otp:
                    for j in range(SR):
                        pt = otp.tile([P, 18], f32, tag="tp", name="ptof")
                        nc.tensor.transpose(pt[:], offc[:, j, :],
                                            id_sb[:18, :18])
                        nc.scalar.activation(ot[:, j, :], pt[:], AF.Copy)

                for g in range(8):
                    dyT = ot[:, 4 * g:4 * g + 4, 0:18:2]
                    dxT = ot[:, 4 * g:4 * g + 4, 1:18:2]

                    def tmp(nm):
                        return dtm.tile([P, 36], f32, tag=nm, name=nm)

                    # ---- sample coords, clamped floors, fracs ----
                    ys = tmp("ys")
                    nc.vector.tensor_tensor(out=ys[:], in0=dyT, in1=ky_sb[:],
                                            op=ALU.add)
                    nc.vector.tensor_scalar(
                        out=ys[:], in0=ys[:], scalar1=mt_sb[:, 1:2],
                        scalar2=float(4 * g), op0=ALU.add, op1=ALU.add)
                    by = tmp("by")
                    nc.vector.tensor_scalar(out=by[:], in0=ys[:],
                                            scalar1=-0.5, scalar2=MAGIC,
                                            op0=ALU.add, op1=ALU.add)
                    nc.vector.tensor_scalar(out=by[:], in0=by[:],
                                            scalar1=-MAGIC, scalar2=None,
                                            op0=ALU.add)
                    nc.vector.tensor_scalar(out=by[:], in0=by[:],
                                            scalar1=-1.0, scalar2=127.0,
                                            op0=ALU.max, op1=ALU.min)
                    fy = tmp("fy")
                    nc.vector.tensor_tensor(out=fy[:], in0=ys[:], in1=by[:],
                                            op=ALU.subtract)
                    nc.vector.tensor_scalar(out=fy[:], in0=fy[:],
                                            scalar1=0.0, scalar2=1.0,
                                            op0=ALU.max, op1=ALU.min)
                    wy0 = tmp("wy0")
                    nc.vector.tensor_scalar(out=wy0[:], in0=fy[:],
                                            scalar1=-1.0, scalar2=1.0,
                                            op0=ALU.mult, op1=ALU.add)

                    xs = tmp("xs")
                    nc.vector.tensor_tensor(out=xs[:], in0=dxT, in1=xk_sb[:],
                                            op=ALU.add)
                    bx = tmp("bx")
                    nc.vector.tensor_scalar(out=bx[:], in0=xs[:],
                                            scalar1=-0.5, scalar2=MAGIC,
                                            op0=ALU.add, op1=ALU.add)
                    nc.vector.tensor_scalar(out=bx[:], in0=bx[:],
                                            scalar1=-MAGIC, scalar2=None,
                                            op0=ALU.add)
                    nc.vector.tensor_scalar(out=bx[:], in0=bx[:],
                                            scalar1=-1.0, scalar2=127.0,
                                            op0=ALU.max, op1=ALU.min)
                    fx = tmp("fx")
                    nc.vector.tensor_tensor(out=fx[:], in0=xs[:], in1=bx[:],
                                            op=ALU.subtract)
                    nc.vector.tensor_scalar(out=fx[:], in0=fx[:],
                                            scalar1=0.0, scalar2=1.0,
                                            op0=ALU.max, op1=ALU.min)
                    wx0 = tmp("wx0")
                    nc.vector.tensor_scalar(out=wx0[:], in0=fx[:],
                                            scalar1=-1.0, scalar2=1.0,
                                            op0=ALU.mult, op1=ALU.add)

                    # corner weights [128, 36, 4]: (y0x0, y0x1, y1x0, y1x1)
                    wv = dix.tile([P, 36, 4], f32, tag="wv", name="wv")
                    nc.vector.tensor_tensor(out=wv[:, :, 0], in0=wy0[:],
                                            in1=wx0[:], op=ALU.mult)
                    nc.vector.tensor_tensor(out=wv[:, :, 1], in0=wy0[:],
                                            in1=fx[:], op=ALU.mult)
                    nc.vector.tensor_tensor(out=wv[:, :, 2], in0=fy[:],
                                            in1=wx0[:], op=ALU.mult)
                    nc.vector.tensor_tensor(out=wv[:, :, 3], in0=fy[:],
                                            in1=fx[:], op=ALU.mult)

                    # gather indices into padded grid: (by+1+yc)*GP + bx+1
                    it = tmp("it")
                    nc.vector.tensor_scalar(out=it[:], in0=by[:],
                                            scalar1=float(GP), scalar2=None,
                                            op0=ALU.mult)
                    nc.vector.tensor_tensor(out=it[:], in0=it[:], in1=bx[:],
                                            op=ALU.add)
                    # gather indices (int32, baseline-style conversion):
                    # pair base rows (by+1+yc)*GP + bx+1 of the padded grid
                    ix32 = dix.tile([P, 36, 2], mybir.dt.int32, tag="ix",
                                    name="ix")
                    nc.vector.tensor_scalar(out=ix32[:, :, 0], in0=it[:],
                                            scalar1=float(GP + 1),
                                            scalar2=None, op0=ALU.add)
                    nc.vector.tensor_scalar(out=ix32[:, :, 1], in0=it[:],
                                            scalar1=float(2 * GP + 1),
                                            scalar2=None, op0=ALU.add)

                    # ---- per-tap: x-pair gathers (4 jj x 2 yc) + combine --
                    pso = [dps.tile([P, 512], f32, tag=f"acc{oj}",
                                    name=f"acc{oj}") for oj in range(2)]
                    pair = fpp[d]
                    for k in range(9):
                        rg = dga.tile([P, 8, 512], bf16, tag="rg", name="rg")
                        for jj in range(4):
                            col = jj * 9 + k
                            for yc in range(2):
                                nc.gpsimd.indirect_dma_start(
                                    out=rg[:, yc * 4 + jj, :],
                                    out_offset=None,
                                    in_=pair,
                                    in_offset=bass.IndirectOffsetOnAxis(
                                        ap=ix32[:, col, yc:yc + 1], axis=0),
                                )

                        tpB = tps.tile([P, 2, 512], bf16, tag="tpB",
                                       name="tpB")
                        ptc = [tpB[:, cc, :] for cc in range(2)]
                        for j in range(4):
                            col = j * 9 + k
                            m0 = dtm.tile([P, C], bf16, tag="m0", name="m0")
                            nc.scalar.activation(
                                m0[:], rg[:, j, 0:C], AF.Copy,
                                scale=wv[:, col, 0:1])
                            m1 = dtm.tile([P, C], bf16, tag="m1", name="m1")
                            nc.vector.tensor_scalar(
                                out=m1[:], in0=rg[:, j, C:2 * C],
                                scalar1=wv[:, col, 1:2], scalar2=None,
                                op0=ALU.mult)
                            m2 = dtm.tile([P, C], bf16, tag="m2", name="m2")
                            nc.scalar.activation(
                                m2[:], rg[:, 4 + j, 0:C], AF.Copy,
                                scale=wv[:, col, 2:3])
                            m3 = dtm.tile([P, C], bf16, tag="m3", name="m3")
                            nc.vector.tensor_scalar(
                                out=m3[:], in0=rg[:, 4 + j, C:2 * C],
                                scalar1=wv[:, col, 3:4], scalar2=None,
                                op0=ALU.mult)
                            nc.vector.tensor_tensor(out=m0[:], in0=m0[:],
                                                    in1=m1[:], op=ALU.add)
                            nc.vector.tensor_tensor(out=m2[:], in0=m2[:],
                                                    in1=m3[:], op=ALU.add)
                            nc.vector.tensor_tensor(out=m0[:], in0=m0[:],
                                                    in1=m2[:], op=ALU.add)
                            for cc in range(2):
                                nc.tensor.transpose(
                                    ptc[cc][:, j * P:(j + 1) * P],
                                    m0[:, cc * P:cc * P + P],
                                    idb_sb[:])
                        st = [dst.tile([P, 4 * P], bf16, tag=f"st{cc}",
                                       name=f"st{cc}") for cc in range(2)]
                        for cc in range(2):
                            nc.scalar.activation(st[cc][:], ptc[cc],
                                                 AF.Copy)
                        for cc in range(2):
                            for oj in range(2):
                                nc.tensor.matmul(
                                    pso[oj][:],
                                    wd_sb[:, (k * 2 + cc) * 2 + oj, :],
                                    st[cc][:],
                                    start=(k == 0 and cc == 0),
                                    stop=(k == 8 and cc == 1))
                    for oj in range(2):
                        nc.scalar.activation(
                            dcnout[d][oj][:, g * 512:(g + 1) * 512],
                            pso[oj][:], AF.Copy)

        # ============== fuse: 1x1 conv 768 -> 256 ==============
        with tc.tile_pool(name="fw", bufs=1) as fw, \
                tc.tile_pool(name="fo", bufs=3) as fo, \
                tc.tile_pool(name="fp", bufs=4, space="PSUM") as fp:
            wf_sb = fw.tile([P, 6 * 2, P], bf16)
            nc.gpsimd.dma_start(out=wf_sb[:], in_=wfl[:])
            f1_sb = [fw.tile([P, SR * W], bf16, tag=f"f1_{j}", name=f"f1_{j}")
                     for j in range(2)]
            for j in range(2):
                nc.gpsimd.dma_start(out=f1_sb[j][:], in_=ft1s[j, :, :])
            frs = [f1_sb[0], f1_sb[1], dcnout[0][0], dcnout[0][1],
                   dcnout[1][0], dcnout[1][1]]
            for nb in range(8):
                sl = slice(nb * 512, (nb + 1) * 512)
                for oj in range(2):
                    ps = fp.tile([P, 512], f32, tag="fps", name="psf")
                    for cc in range(6):
                        nc.tensor.matmul(
                            ps[:], wf_sb[:, cc * 2 + oj, :],
                            frs[cc][:, sl],
                            start=(cc == 0), stop=(cc == 5))
                    ob = fo.tile([P, 512], f32, tag="ob", name="ob")
                    nc.scalar.activation(ob[:], ps[:], AF.Identity,
                                         bias=bf_sb[:, oj:oj + 1])
                    nc.sync.dma_start(out=out[oj, :, sl], in_=ob[:])

    if split_waits:
        _split_sync_waits(nc)
    return nc


# --------------------------------------------------------------------------
# Host-side input prep / output assembly
# --------------------------------------------------------------------------
def prep_inputs(ft_1, ft_2, ft_3, w1, b1, w2, b2, w3, b3, w_dcn2, w_dcn3,
                w_fuse, b_fuse):
    import ml_dtypes
    bfnp = ml_dtypes.bfloat16

    ft_1, ft_2, ft_3 = (np.asarray(a, np.float32) for a in (ft_1, ft_2, ft_3))
    combined = np.concatenate([ft_1, ft_2, ft_3], axis=1)  # [B, 768, H, W]

    def conv_lhsT(wt, n_cc, n_oj, mm):
        # [Cout, Cin, 3, 3] -> [128, n_cc*9*n_oj, mm]
        wt = np.asarray(wt, np.float32)
        a = wt.reshape(n_oj, mm, n_cc, P, 3, 3)
        a = a.transpose(3, 2, 4, 5, 0, 1)  # [ci, cc, ty, tx, oj, o]
        return np.ascontiguousarray(a.reshape(P, n_cc * 9 * n_oj, mm))

    w1l = conv_lhsT(w1, 6, 2, P)
    w2l = conv_lhsT(w2, 2, 2, P)
    # conv3: pad output channels 36 -> 50 so each dcn's 18 offset channels
    # start at a legal partition offset (0 and 32)
    w3p = np.zeros((50, 768 // 3, 3, 3), np.float32)
    w3a = np.asarray(w3, np.float32)
    w3p[0:18] = w3a[0:18]
    w3p[32:50] = w3a[18:36]
    w3l = conv_lhsT(w3p, 2, 1, 50)

    def dcn_lhsT(wt):
        # [256, 256, 3, 3] -> [128ci, (k, cc, oj), 128o]
        wt = np.asarray(wt, np.float32)
        a = wt.reshape(2, P, 2, P, 9)        # [ojb, o, cc, ci, k]
        a = a.transpose(3, 4, 2, 0, 1)       # [ci, k, cc, oj, o]
        return np.ascontiguousarray(a.reshape(P, 9 * 2 * 2, P)).astype(bfnp)

    wd2l = dcn_lhsT(w_dcn2)
    wd3l = dcn_lhsT(w_dcn3)

    wf = np.asarray(w_fuse, np.float32).reshape(256, 768)
    a = wf.reshape(2, P, 6, P).transpose(3, 2, 0, 1)  # [ci, cc, oj, o]
    wfl = np.ascontiguousarray(a.reshape(P, 6 * 2, P)).astype(bfnp)

    b1p = np.ascontiguousarray(np.asarray(b1, np.float32).reshape(2, P).T)
    b2p = np.ascontiguousarray(np.asarray(b2, np.float32).reshape(2, P).T)
    b3p = np.zeros((50, 1), np.float32)
    b3a = np.asarray(b3, np.float32).reshape(36)
    b3p[0:18, 0] = b3a[0:18]
    b3p[32:50, 0] = b3a[18:36]
    bfp = np.ascontiguousarray(np.asarray(b_fuse, np.float32).reshape(2, P).T)

    jj, kk = np.meshgrid(np.arange(4), np.arange(9), indexing="ij")
    kyrow = (jj + kk // 3 - 1).astype(np.float32).reshape(1, 36)
    kyrow = np.ascontiguousarray(np.broadcast_to(kyrow, (P, 36)))
    xkx = (np.arange(P)[:, None] + (kk % 3 - 1).reshape(1, 36)).astype(
        np.float32)
    xkx = np.ascontiguousarray(xkx)
    ident = np.eye(P, dtype=np.float32)
    identb = np.eye(P, dtype=np.float32).astype(bfnp)

    # padded pixel-major gather sources (zero border, bf16), with each
    # DRAM row holding the x-adjacent pixel PAIR (r, r+1) so one indirect
    # descriptor fetches all the data for one (pixel, tap, y-corner)
    def padgrid(ft):
        p = np.zeros((B, GP, GP, C), dtype=bfnp)
        p[:, 1:1 + H, 1:1 + W, :] = ft.transpose(0, 2, 3, 1).astype(bfnp)
        p = p.reshape(B, GR, C)
        return np.ascontiguousarray(
            np.concatenate([p[:, :-1, :], p[:, 1:, :]], axis=2))

    fp2p = padgrid(ft_2)
    fp3p = padgrid(ft_3)

    in_maps = []
    for core in range(NCORES):
        b, s = divmod(core, 4)
        r0 = s * SR
        xin = np.zeros((6, P, 38, WP), np.float32)
        lo, hi = r0 - 3, r0 + SR + 3            # conv1 input rows
        vlo, vhi = max(lo, 0), min(hi, H)
        xin[:, :, vlo - lo:vhi - lo, XOFF:XOFF + W] = (
            combined[b, :, vlo:vhi, :].reshape(6, P, vhi - vlo, W))
        ft1s = np.ascontiguousarray(
            ft_1[b, :, r0:r0 + SR, :].reshape(2, P, SR * W)).astype(bfnp)
        mt = np.zeros((P, 2), np.float32)
        mt[:, 1] = r0

        def bmask(rows, rlo):
            m = np.zeros((rows, WP), np.float32)
            for i in range(rows):
                if 0 <= rlo + i < H:
                    m[i, XOFF:XOFF + W] = 1.0
            return np.ascontiguousarray(
                np.broadcast_to(m.reshape(1, rows * WP), (P, rows * WP)))

        m1 = bmask(36, r0 - 2)
        m2 = bmask(34, r0 - 1)
        xinf = np.zeros((6, P, 38 * WP + 2), np.float32)
        xinf[:, :, 1:1 + 38 * WP] = xin.reshape(6, P, 38 * WP)
        fp2c = fp2p[b]
        fp3c = fp3p[b]
        in_maps.append({
            "xin": xinf,
            "ft1s": ft1s, "fp2p": fp2c, "fp3p": fp3c,
            "w1l": w1l, "w2l": w2l, "w3l": w3l,
            "wd2l": wd2l, "wd3l": wd3l, "wfl": wfl,
            "b1": b1p, "b2": b2p, "b3": b3p, "bf": bfp,
            "kyrow": kyrow, "xkx": xkx, "meta": mt, "ident": ident,
            "identb": identb,
            "m1m": m1, "m2m": m2,
        })
    return in_maps


def assemble_output(results):
    full = np.empty((B, C, H, W), np.float32)
    for core in range(NCORES):
        b, s = divmod(core, 4)
        r0 = s * SR
        o = results[core]["out"]            # [2, 128, SR*W]
        for oj in range(2):
            full[b, oj * P:(oj + 1) * P, r0:r0 + SR, :] = o[oj].reshape(
                P, SR, W)
    return full


_CACHED_NC = None


def kernel(**inputs) -> np.ndarray:
    global _CACHED_NC
    in_maps = prep_inputs(**inputs)
    if _CACHED_NC is None:
        _CACHED_NC = build_program()
    res = run_bass_kernel_spmd(_CACHED_NC, in_maps, list(range(NCORES)))
    return assemble_output(res.results)


if __name__ == "__main__":
    import json
    rng = np.random.default_rng(0)
    print("building program (syntax check)...")
    nc = build_program()
    print("instructions:",
          sum(len(bb.instructions) for bb in nc.m.functions[0].blocks))


# revision 36
# speedup vs baseline: 1.6084x; 1.1766x over previous
"""Trainium2 Bass kernel for nn_DeformableFusion.

Pipeline (reference): concat(ft1,ft2,ft3) -> conv3x3(768->256)+relu ->
conv3x3(256->256)+relu -> conv3x3(256->36) = offsets -> two deformable
convs (ft_2, ft_3) -> concat(ft_1, a2, a3) -> conv1x1(768->256).

Sharding: 8 cores = 2 samples x 4 row-strips of 32 rows. Each core gets
zero-padded strip inputs for the conv chain plus full PADDED pixel-major
bf16 copies of ft_2/ft_3 (130x130 grid, zero border) so the deformable
gather needs no halo exchange and no validity masks: clamped corner
coords land on zero pixels, so out-of-image corners contribute 0
automatically.

Conv chain runs in float32r (full-rate ~TF32); the offset path must stay
f32-precision (bf16 there pushes rel err past 1e-2). The deformable path
runs in bf16: each indirect DMA descriptor fetches one x-adjacent pixel
PAIR (1KB) from a pair-materialized DRAM layout ([GR-1, 2C], row r =
pixels r,r+1), halving both gather calls and bytes vs per-corner f32
gathers. Corner weights apply via clamp-lerp fractions (per-partition
TensorScalar at 4x bf16 rate + Activation scale copies), corners sum on
DVE, transpose to channel-major on PE, dcn + fuse matmuls in bf16.
"""
import sys
from contextlib import ExitStack

sys.path.insert(0, "/opt/trn_rl_repo")

import numpy as np

import concourse.bass as bass
import concourse.mybir as mybir
from concourse.bass_utils import run_bass_kernel_spmd
from concourse.tile import TileContext

f32 = mybir.dt.float32
f32r = mybir.dt.float32r  # TF32-like on HW (rel ~1.6e-3): 4x matmul rate vs fp32
bf16 = mybir.dt.bfloat16
i16 = mybir.dt.int16
AF = mybir.ActivationFunctionType
ALU = mybir.AluOpType

P = 128
B, C, H, W = 2, 256, 128, 128
KK = 9
NCORES = 8
SR = 32          # strip rows per core
WP = 134         # padded grid width (x in [-3, 131))
XOFF = 3         # image x -> padded col offset
NB = 402         # conv matmul moving-block size (3 rows of WP)
HW = H * W
GP = 130         # padded gather grid (y', x' in [0, 130); zero border)
GR = GP * GP     # 16900 rows per (batch, tensor)
MAGIC = 12582912.0  # 1.5 * 2**23, fp32 round-to-int magic


# --------------------------------------------------------------------------
# Walrus in this toolchain rejects instructions carrying more than ~2 sync
# waits ("Too many sync wait commands" on the Tile tail Drain). Spread the
# global-clock waits one-per-NOP before the drain.
# --------------------------------------------------------------------------
def _patch_tile_drain():
    import re

    import bass_rust
    import concourse.tile as tile_mod

    ScopedClock = bass_rust.ScopedClock
    VectorClock = bass_rust.VectorClock

    def _vc_ticks(vc):
        m = re.search(r"VectorClock\(\[(.*)\]\)", repr(vc))
        body = m.group(1).strip()
        return [int(t) for t in body.split(",")] if body else []

    def _drain_and_barrier(self, tick_clock, wait_clock):
        ticks = _vc_ticks(tick_clock.global_clock)
        for proc, tick in enumerate(ticks):
            if tick <= 0:
                continue
            single = [0] * len(ticks)
            single[proc] = tick
            nop = self.nc.sync.nop(nofuse=True, hint=f"drain_wait_p{proc}")
            wait_clock.add_sem_waits(
                nop.ins, ScopedClock({None: VectorClock(single)})
            )
        drain_inst = self.nc.sync.drain()
        wait_clock.add_sem_waits(
            drain_inst.ins,
            ScopedClock({None: tick_clock.global_clock}),
            ScopedClock({None: tick_clock.global_clock.copy()}),
        )
        self.nc.all_engine_barrier()
        assert self.sems is not None
        popped = self.nc._tile_sem_poison_stack.pop()
        assert popped is self._sem_poison
        self.nc.clear_and_free_semaphores(list(self.sems.allocated().values()))
        self.nc.all_engine_barrier()

    tile_mod.TileContext._drain_and_barrier = _drain_and_barrier


_patch_tile_drain()


def _split_sync_waits(nc, cap=1):
    """Walrus in this toolchain caps sync waits per instruction. Hoist
    excess waits onto same-engine NoOps inserted immediately before the
    overloaded instruction (engines are in-order, so waiting earlier on
    the same engine is always safe in this straight-line program)."""
    n = 0
    for bb in nc.m.functions[0].blocks:
        insts = bb.instructions
        i = 0
        while i < len(insts):
            inst = insts[i]
            si = inst.sync_info
            waits = si.on_wait if si is not None else None
            if waits and len(waits) > cap:
                excess = waits[cap:]
                del waits[cap:]
                for j in range(0, len(excess), cap):
                    nop = mybir.InstNoOp(
                        name=f"I-waitsplit-{n}", ins=[], outs=[],
                        engine=inst.engine,
                        sync_info=mybir.SyncInfo(
                            on_wait=excess[j:j + cap], on_update=[]),
                        bass_nofuse=True,
                    )
                    n += 1
                    insts.insert(i, nop)
                    i += 1
            i += 1
    return n


# --------------------------------------------------------------------------
# Device program
# --------------------------------------------------------------------------
def _conv_pass(nc, ppool, in_tiles, w_sb, n_cc, n_oj, taps,
               rows_out, out_write):
    """Shift-accumulation 3x3 conv over the flat padded grid."""
    total = rows_out * WP
    m = w_sb.shape[-1]
    p0 = 0
    while p0 < total:
        size = min(NB, total - p0)
        for oj in range(n_oj):
            ps = ppool.tile([P, 512], f32, tag="cpsum", name="psc")
            nmm = n_cc * taps
            i = 0
            for cc in range(n_cc):
                for t in range(taps):
                    ty, tx = t // 3, t % 3
                    roff = 1 + p0 + ty * WP + (tx - 1)
                    nc.tensor.matmul(
                        ps[:m, :size],
                        w_sb[:, ((cc * taps + t) * n_oj + oj), :],
                        in_tiles[cc][:, roff:roff + size],
                        start=(i == 0),
                        stop=(i == nmm - 1),
                    )
                    i += 1
            out_write(oj, p0, size, ps)
        p0 += size


def build_program(split_waits=True):
    nc = bass.Bass("TRN2", target_bir_lowering=False, debug=False,
                   num_devices=NCORES)

    def din(name, shape, dtype=f32):
        return nc.dram_tensor(name, shape, dtype, kind="ExternalInput").ap()

    # conv-chain input strip: 38 rows x 134 cols, zero-padded, 6 c-chunks
    xin = din("xin", [6, P, 38 * WP + 2])
    ft1s = din("ft1s", [2, P, SR * W], bf16)    # fuse input strip (bf16)
    fpp = [din("fp2p", [GR - 1, 2 * C], bf16),
           din("fp3p", [GR - 1, 2 * C], bf16)]
    w1l = din("w1l", [P, 6 * 9 * 2, P])
    w2l = din("w2l", [P, 2 * 9 * 2, P])
    w3l = din("w3l", [P, 2 * 9 * 1, 50])
    wdl = [din("wd2l", [P, 9 * 2 * 2, P], bf16),
           din("wd3l", [P, 9 * 2 * 2, P], bf16)]
    wfl = din("wfl", [P, 6 * 2, P], bf16)
    b1 = din("b1", [P, 2])
    b2 = din("b2", [P, 2])
    b3 = din("b3", [50, 1])
    bf = din("bf", [P, 2])
    kyrow = din("kyrow", [P, 36])   # (j,k): j + ky[k]
    xkx = din("xkx", [P, 36])       # (j,k): x(part) + kx[k]
    meta = din("meta", [P, 2])      # col1: r0 (col0 unused)
    m1m = din("m1m", [P, 36 * WP])  # h1 image-boundary mask (rows+cols)
    m2m = din("m2m", [P, 34 * WP])  # h2 image-boundary mask
    ident = din("ident", [P, P])
    identb = din("identb", [P, P], bf16)
    out = nc.dram_tensor("out", [2, P, SR * W], f32, kind="ExternalOutput").ap()

    with TileContext(nc) as tc, ExitStack() as es:
        cst = es.enter_context(tc.tile_pool(name="cst", bufs=1))
        ky_sb = cst.tile([P, 36], f32)
        nc.sync.dma_start(out=ky_sb[:], in_=kyrow[:])
        xk_sb = cst.tile([P, 36], f32)
        nc.sync.dma_start(out=xk_sb[:], in_=xkx[:])
        mt_sb = cst.tile([P, 2], f32)
        nc.sync.dma_start(out=mt_sb[:], in_=meta[:])
        id_sb = cst.tile([P, P], f32)
        nc.sync.dma_start(out=id_sb[:], in_=ident[:])
        idb_sb = cst.tile([P, P], bf16)
        nc.sync.dma_start(out=idb_sb[:], in_=identb[:])
        b1_sb = cst.tile([P, 2], f32)
        nc.sync.dma_start(out=b1_sb[:], in_=b1[:])
        b2_sb = cst.tile([P, 2], f32)
        nc.sync.dma_start(out=b2_sb[:], in_=b2[:])
        b3_sb = cst.tile([50, 1], f32)
        nc.sync.dma_start(out=b3_sb[:], in_=b3[:])
        bf_sb = cst.tile([P, 2], f32)
        nc.sync.dma_start(out=bf_sb[:], in_=bf[:])

        p_off = es.enter_context(tc.tile_pool(name="p_off", bufs=1))
        off = p_off.tile([50, 32 * WP], f32)

        # ============== conv chain (h1/h2 live only here) ==============
        with tc.tile_pool(name="p_h", bufs=1) as p_h:
            h1 = [p_h.tile([P, 36 * WP + 2], f32r, tag=f"h1_{j}",
                           name=f"h1_{j}") for j in range(2)]
            h2 = [p_h.tile([P, 34 * WP + 2], f32r, tag=f"h2_{j}",
                           name=f"h2_{j}") for j in range(2)]

            # ---- conv1: 768 -> 256, relu ----
            with tc.tile_pool(name="c1w", bufs=1) as c1w, \
                    tc.tile_pool(name="c1x", bufs=2) as c1x, \
                    tc.tile_pool(name="c1p", bufs=4, space="PSUM") as c1p:
                w1_sb = c1w.tile([P, 6 * 9 * 2, P], f32r)
                nc.gpsimd.dma_start(out=w1_sb[:], in_=w1l[:])

                total = 36 * WP
                p0 = 0
                while p0 < total:
                    size = min(NB, total - p0)
                    rb = p0 // WP          # block starts at a row boundary
                    xts = []
                    for cc in range(6):
                        xt = c1x.tile([P, 5 * WP + 2], f32r, tag=f"xt{cc}",
                                      name=f"xt{cc}")
                        nc.gpsimd.dma_start(
                            out=xt[:],
                            in_=xin[cc, :, rb * WP:rb * WP + 5 * WP + 2])
                        xts.append(xt)

                    for oj in range(2):
                        ps = c1p.tile([P, 512], f32, tag="cpsum", name="ps1")
                        i = 0
                        for cc in range(6):
                            for t in range(9):
                                ty, tx = t // 3, t % 3
                                roff = 1 + ty * WP + (tx - 1)
                                nc.tensor.matmul(
                                    ps[:, :size],
                                    w1_sb[:, (cc * 9 + t) * 2 + oj, :],
                                    xts[cc][:, roff:roff + size],
                                    start=(i == 0), stop=(i == 53))
                                i += 1
                        nc.scalar.activation(
                            h1[oj][:, 1 + p0:1 + p0 + size], ps[:, :size],
                            AF.Relu, bias=b1_sb[:, oj:oj + 1])
                    p0 += size

            # zero h1 outside the image (reference pads h1 with zeros)
            with tc.tile_pool(name="pm1", bufs=1) as pm1:
                m1_sb = pm1.tile([P, 36 * WP], f32r)
                nc.gpsimd.dma_start(out=m1_sb[:], in_=m1m[:])
                for oj in range(2):
                    nc.vector.tensor_tensor(
                        out=h1[oj][:, 1:1 + 36 * WP],
                        in0=h1[oj][:, 1:1 + 36 * WP],
                        in1=m1_sb[:], op=ALU.mult)
                    nc.vector.tensor_copy(out=h1[oj][:, 0:1],
                                          in_=m1_sb[:, 0:1])
                    nc.vector.tensor_copy(out=h1[oj][:, 1 + 36 * WP:],
                                          in_=m1_sb[:, 0:1])

            # ---- conv2: 256 -> 256, relu ----
            with tc.tile_pool(name="c2w", bufs=1) as c2w, \
                    tc.tile_pool(name="c2p", bufs=4, space="PSUM") as c2p:
                w2_sb = c2w.tile([P, 2 * 9 * 2, P], f32r)
                nc.gpsimd.dma_start(out=w2_sb[:], in_=w2l[:])

                def h2_write(oj, p0, size, ps):
                    nc.scalar.activation(
                        h2[oj][:, 1 + p0:1 + p0 + size], ps[:, :size],
                        AF.Relu, bias=b2_sb[:, oj:oj + 1])

                _conv_pass(nc, c2p, h1, w2_sb, 2, 2, 9, 34, h2_write)

            # zero h2 outside the image
            with tc.tile_pool(name="pm2", bufs=1) as pm2:
                m2_sb = pm2.tile([P, 34 * WP], f32r)
                nc.gpsimd.dma_start(out=m2_sb[:], in_=m2m[:])
                for oj in range(2):
                    nc.vector.tensor_tensor(
                        out=h2[oj][:, 1:1 + 34 * WP],
                        in0=h2[oj][:, 1:1 + 34 * WP],
                        in1=m2_sb[:], op=ALU.mult)
                    nc.vector.tensor_copy(out=h2[oj][:, 0:1],
                                          in_=m2_sb[:, 0:1])
                    nc.vector.tensor_copy(out=h2[oj][:, 1 + 34 * WP:],
                                          in_=m2_sb[:, 0:1])

            # ---- conv3: 256 -> 36 (offsets) ----
            with tc.tile_pool(name="c3w", bufs=1) as c3w, \
                    tc.tile_pool(name="c3p", bufs=4, space="PSUM") as c3p:
                w3_sb = c3w.tile([P, 2 * 9 * 1, 50], f32r)
                nc.gpsimd.dma_start(out=w3_sb[:], in_=w3l[:])

                def off_write(oj, p0, size, ps):
                    nc.scalar.activation(
                        off[:, p0:p0 + size], ps[:50, :size],
                        AF.Identity, bias=b3_sb[:, 0:1])

                _conv_pass(nc, c3p, h2, w3_sb, 2, 1, 9, 32, off_write)

        # ============== deformable convs ==============
        p_do = es.enter_context(tc.tile_pool(name="p_do", bufs=1))
        dcnout = [[p_do.tile([P, SR * W], bf16, tag=f"dcn{d}_{oj}",
                             name=f"dcn{d}_{oj}")
                   for oj in range(2)] for d in range(2)]
        offv = off[:].rearrange("p (r w) -> p r w", w=WP)



        for d in range(2):
            with tc.tile_pool(name="dwp", bufs=1) as dwp, \
                    tc.tile_pool(name="dix", bufs=2) as dix, \
                    tc.tile_pool(name="dga", bufs=2) as dga, \
                    tc.tile_pool(name="dst", bufs=2) as dst, \
                    tc.tile_pool(name="dtm", bufs=3) as dtm, \
                    tc.tile_pool(name="dps", bufs=2, space="PSUM") as dps, \
                    tc.tile_pool(name="tps", bufs=2, space="PSUM") as tps:
                wd_sb = dwp.tile([P, 9 * 2 * 2, P], bf16, tag="wd", name="wd")
                nc.gpsimd.dma_start(out=wd_sb[:], in_=wdl[d][:])

                # compact offsets [18, 32, 128], transpose to [128, 32, 18]
                offc = dwp.tile([18, SR, W], f32, tag="offc", name="offc")
                nc.vector.tensor_copy(
                    out=offc[:],
                    in_=offv[32 * d:32 * d + 18, :, XOFF:XOFF + W])
                ot = dwp.tile([P, SR, 18], f32, tag="ot", name="ot")
                with tc.tile_pool(name="otp", bufs=2, space="PSUM") as otp:
                    for j in range(SR):
                        pt = otp.tile([P, 18], f32, tag="tp", name="ptof")
                        nc.tensor.transpose(pt[:], offc[:, j, :],
                                            id_sb[:18, :18])
                        nc.scalar.activation(ot[:, j, :], pt[:], AF.Copy)

                for g in range(8):
                    dyT = ot[:, 4 * g:4 * g + 4, 0:18:2]
                    dxT = ot[:, 4 * g:4 * g + 4, 1:18:2]

                    def tmp(nm):
                        return dtm.tile([P, 36], f32, tag=nm, name=nm)

                    # ---- sample coords, clamped floors, fracs ----
                    ys = tmp("ys")
                    nc.vector.tensor_tensor(out=ys[:], in0=dyT, in1=ky_sb[:],
                                            op=ALU.add)
                    nc.vector.tensor_scalar(
                        out=ys[:], in0=ys[:], scalar1=mt_sb[:, 1:2],
                        scalar2=float(4 * g), op0=ALU.add, op1=ALU.add)
                    by = tmp("by")
                    nc.vector.tensor_scalar(out=by[:], in0=ys[:],
                                            scalar1=-0.5, scalar2=MAGIC,
                                            op0=ALU.add, op1=ALU.add)
                    nc.vector.tensor_scalar(out=by[:], in0=by[:],
                                            scalar1=-MAGIC, scalar2=None,
                                            op0=ALU.add)
                    nc.vector.tensor_scalar(out=by[:], in0=by[:],
                                            scalar1=-1.0, scalar2=127.0,
                                            op0=ALU.max, op1=ALU.min)
                    fy = tmp("fy")
                    nc.vector.tensor_tensor(out=fy[:], in0=ys[:], in1=by[:],
                                            op=ALU.subtract)
                    nc.vector.tensor_scalar(out=fy[:], in0=fy[:],
                                            scalar1=0.0, scalar2=1.0,
                                            op0=ALU.max, op1=ALU.min)
                    wy0 = tmp("wy0")
                    nc.vector.tensor_scalar(out=wy0[:], in0=fy[:],
                                            scalar1=-1.0, scalar2=1.0,
                                            op0=ALU.mult, op1=ALU.add)

                    xs = tmp("xs")
                    nc.vector.tensor_tensor(out=xs[:], in0=dxT, in1=xk_sb[:],
                                            op=ALU.add)
                    bx = tmp("bx")
                    nc.vector.tensor_scalar(out=bx[:], in0=xs[:],
                                            scalar1=-0.5, scalar2=MAGIC,
                                            op0=ALU.add, op1=ALU.add)
                    nc.vector.tensor_scalar(out=bx[:], in0=bx[:],
                                            scalar1=-MAGIC, scalar2=None,
                                            op0=ALU.add)
                    nc.vector.tensor_scalar(out=bx[:], in0=bx[:],
                                            scalar1=-1.0, scalar2=127.0,
                                            op0=ALU.max, op1=ALU.min)
                    fx = tmp("fx")
                    nc.vector.tensor_tensor(out=fx[:], in0=xs[:], in1=bx[:],
                                            op=ALU.subtract)
                    nc.vector.tensor_scalar(out=fx[:], in0=fx[:],
                                            scalar1=0.0, scalar2=1.0,
                                            op0=ALU.max, op1=ALU.min)
                    wx0 = tmp("wx0")
                    nc.vector.tensor_scalar(out=wx0[:], in0=fx[:],
                                            scalar1=-1.0, scalar2=1.0,
                                            op0=ALU.mult, op1=ALU.add)

                    # corner weights [128, 36, 4]: (y0x0, y0x1, y1x0, y1x1)
                    wv = dix.tile([P, 36, 4], f32, tag="wv", name="wv")
                    nc.vector.tensor_tensor(out=wv[:, :, 0], in0=wy0[:],
                                            in1=wx0[:], op=ALU.mult)
                    nc.vector.tensor_tensor(out=wv[:, :, 1], in0=wy0[:],
                                            in1=fx[:], op=ALU.mult)
                    nc.vector.tensor_tensor(out=wv[:, :, 2], in0=fy[:],
                                            in1=wx0[:], op=ALU.mult)
                    nc.vector.tensor_tensor(out=wv[:, :, 3], in0=fy[:],
                                            in1=fx[:], op=ALU.mult)

                    # gather indices into padded grid: (by+1+yc)*GP + bx+1
                    it = tmp("it")
                    nc.vector.tensor_scalar(out=it[:], in0=by[:],
                                            scalar1=float(GP), scalar2=None,
                                            op0=ALU.mult)
                    nc.vector.tensor_tensor(out=it[:], in0=it[:], in1=bx[:],
                                            op=ALU.add)
                    # gather indices (int32, baseline-style conversion):
                    # pair base rows (by+1+yc)*GP + bx+1 of the padded grid
                    ix32 = dix.tile([P, 36, 2], mybir.dt.int32, tag="ix",
                                    name="ix")
                    nc.vector.tensor_scalar(out=ix32[:, :, 0], in0=it[:],
                                            scalar1=float(GP + 1),
                                            scalar2=None, op0=ALU.add)
                    nc.vector.tensor_scalar(out=ix32[:, :, 1], in0=it[:],
                                            scalar1=float(2 * GP + 1),
                                            scalar2=None, op0=ALU.add)

                    # ---- per-tap: x-pair gathers (4 jj x 2 yc) + combine --
                    pso = [dps.tile([P, 512], f32, tag=f"acc{oj}",
                                    name=f"acc{oj}") for oj in range(2)]
                    pair = fpp[d]
                    for k in range(9):
                        rg = dga.tile([P, 8, 512], bf16, tag="rg", name="rg")
                        for jj in range(4):
                            col = jj * 9 + k
                            for yc in range(2):
                                nc.gpsimd.indirect_dma_start(
                                    out=rg[:, yc * 4 + jj, :],
                                    out_offset=None,
                                    in_=pair,
                                    in_offset=bass.IndirectOffsetOnAxis(
                                        ap=ix32[:, col, yc:yc + 1], axis=0),
                                )

                        tpB = tps.tile([P, 2, 512], bf16, tag="tpB",
                                       name="tpB")
                        ptc = [tpB[:, cc, :] for cc in range(2)]
                        for j in range(4):
                            col = j * 9 + k
                            m0 = dtm.tile([P, C], bf16, tag="m0", name="m0")
                            nc.scalar.activation(
                                m0[:], rg[:, j, 0:C], AF.Copy,
                                scale=wv[:, col, 0:1])
                            m1 = dtm.tile([P, C], bf16, tag="m1", name="m1")
                            nc.vector.tensor_scalar(
                                out=m1[:], in0=rg[:, j, C:2 * C],
                                scalar1=wv[:, col, 1:2], scalar2=None,
                                op0=ALU.mult)
                            m2 = dtm.tile([P, C], bf16, tag="m2", name="m2")
                            nc.scalar.activation(
                                m2[:], rg[:, 4 + j, 0:C], AF.Copy,
                                scale=wv[:, col, 2:3])
                            m3 = dtm.tile([P, C], bf16, tag="m3", name="m3")
                            nc.vector.tensor_scalar(
                                out=m3[:], in0=rg[:, 4 + j, C:2 * C],
                                scalar1=wv[:, col, 3:4], scalar2=None,
                                op0=ALU.mult)
                            nc.vector.tensor_tensor(out=m0[:], in0=m0[:],
                                                    in1=m1[:], op=ALU.add)
                            nc.vector.tensor_tensor(out=m2[:], in0=m2[:],
                                                    in1=m3[:], op=ALU.add)
                            nc.vector.tensor_tensor(out=m0[:], in0=m0[:],
                                                    in1=m2[:], op=ALU.add)
                            for cc in range(2):
                                nc.tensor.transpose(
                                    ptc[cc][:, j * P:(j + 1) * P],
                                    m0[:, cc * P:cc * P + P],
                                    idb_sb[:])
                        st = [dst.tile([P, 4 * P], bf16, tag=f"st{cc}",
                                       name=f"st{cc}") for cc in range(2)]
                        for cc in range(2):
                            nc.scalar.activation(st[cc][:], ptc[cc],
                                                 AF.Copy)
                        for cc in range(2):
                            for oj in range(2):
                                nc.tensor.matmul(
                                    pso[oj][:],
                                    wd_sb[:, (k * 2 + cc) * 2 + oj, :],
                                    st[cc][:],
                                    start=(k == 0 and cc == 0),
                                    stop=(k == 8 and cc == 1))
                    for oj in range(2):
                        nc.scalar.activation(
                            dcnout[d][oj][:, g * 512:(g + 1) * 512],
                            pso[oj][:], AF.Copy)

        # ============== fuse: 1x1 conv 768 -> 256 ==============
        with tc.tile_pool(name="fw", bufs=1) as fw, \
                tc.tile_pool(name="fo", bufs=3) as fo, \
                tc.tile_pool(name="fp", bufs=4, space="PSUM") as fp:
            wf_sb = fw.tile([P, 6 * 2, P], bf16)
            nc.gpsimd.dma_start(out=wf_sb[:], in_=wfl[:])
            f1_sb = [fw.tile([P, SR * W], bf16, tag=f"f1_{j}", name=f"f1_{j}")
                     for j in range(2)]
            for j in range(2):
                nc.gpsimd.dma_start(out=f1_sb[j][:], in_=ft1s[j, :, :])
            frs = [f1_sb[0], f1_sb[1], dcnout[0][0], dcnout[0][1],
                   dcnout[1][0], dcnout[1][1]]
            for nb in range(8):
                sl = slice(nb * 512, (nb + 1) * 512)
                for oj in range(2):
                    ps = fp.tile([P, 512], f32, tag="fps", name="psf")
                    for cc in range(6):
                        nc.tensor.matmul(
                            ps[:], wf_sb[:, cc * 2 + oj, :],
                            frs[cc][:, sl],
                            start=(cc == 0), stop=(cc == 5))
                    ob = fo.tile([P, 512], f32, tag="ob", name="ob")
                    nc.scalar.activation(ob[:], ps[:], AF.Identity,
                                         bias=bf_sb[:, oj:oj + 1])
                    nc.sync.dma_start(out=out[oj, :, sl], in_=ob[:])

    if split_waits:
        _split_sync_waits(nc)
    return nc


# --------------------------------------------------------------------------
# Host-side input prep / output assembly
# --------------------------------------------------------------------------
def prep_inputs(ft_1, ft_2, ft_3, w1, b1, w2, b2, w3, b3, w_dcn2, w_dcn3,
                w_fuse, b_fuse):
    import ml_dtypes
    bfnp = ml_dtypes.bfloat16

    ft_1, ft_2, ft_3 = (np.asarray(a, np.float32) for a in (ft_1, ft_2, ft_3))
    combined = np.concatenate([ft_1, ft_2, ft_3], axis=1)  # [B, 768, H, W]

    def conv_lhsT(wt, n_cc, n_oj, mm, dt=np.float32):
        # [Cout, Cin, 3, 3] -> [128, n_cc*9*n_oj, mm]
        wt = np.asarray(wt, np.float32)
        a = wt.reshape(n_oj, mm, n_cc, P, 3, 3)
        a = a.transpose(3, 2, 4, 5, 0, 1)  # [ci, cc, ty, tx, oj, o]
        return np.ascontiguousarray(
            a.reshape(P, n_cc * 9 * n_oj, mm)).astype(dt)

    w1l = conv_lhsT(w1, 6, 2, P)
    w2l = conv_lhsT(w2, 2, 2, P)
    # conv3: pad output channels 36 -> 50 so each dcn's 18 offset channels
    # start at a legal partition offset (0 and 32)
    w3p = np.zeros((50, 768 // 3, 3, 3), np.float32)
    w3a = np.asarray(w3, np.float32)
    w3p[0:18] = w3a[0:18]
    w3p[32:50] = w3a[18:36]
    w3l = conv_lhsT(w3p, 2, 1, 50)

    def dcn_lhsT(wt):
        # [256, 256, 3, 3] -> [128ci, (k, cc, oj), 128o]
        wt = np.asarray(wt, np.float32)
        a = wt.reshape(2, P, 2, P, 9)        # [ojb, o, cc, ci, k]
        a = a.transpose(3, 4, 2, 0, 1)       # [ci, k, cc, oj, o]
        return np.ascontiguousarray(a.reshape(P, 9 * 2 * 2, P)).astype(bfnp)

    wd2l = dcn_lhsT(w_dcn2)
    wd3l = dcn_lhsT(w_dcn3)

    wf = np.asarray(w_fuse, np.float32).reshape(256, 768)
    a = wf.reshape(2, P, 6, P).transpose(3, 2, 0, 1)  # [ci, cc, oj, o]
    wfl = np.ascontiguousarray(a.reshape(P, 6 * 2, P)).astype(bfnp)

    b1p = np.ascontiguousarray(np.asarray(b1, np.float32).reshape(2, P).T)
    b2p = np.ascontiguousarray(np.asarray(b2, np.float32).reshape(2, P).T)
    b3p = np.zeros((50, 1), np.float32)
    b3a = np.asarray(b3, np.float32).reshape(36)
    b3p[0:18, 0] = b3a[0:18]
    b3p[32:50, 0] = b3a[18:36]
    bfp = np.ascontiguousarray(np.asarray(b_fuse, np.float32).reshape(2, P).T)

    jj, kk = np.meshgrid(np.arange(4), np.arange(9), indexing="ij")
    kyrow = (jj + kk // 3 - 1).astype(np.float32).reshape(1, 36)
    kyrow = np.ascontiguousarray(np.broadcast_to(kyrow, (P, 36)))
    xkx = (np.arange(P)[:, None] + (kk % 3 - 1).reshape(1, 36)).astype(
        np.float32)
    xkx = np.ascontiguousarray(xkx)
    ident = np.eye(P, dtype=np.float32)
    identb = np.eye(P, dtype=np.float32).astype(bfnp)

    # padded pixel-major gather sources (zero border, bf16), with each
    # DRAM row holding the x-adjacent pixel PAIR (r, r+1) so one indirect
    # descriptor fetches all the data for one (pixel, tap, y-corner)
    def padgrid(ft):
        p = np.zeros((B, GP, GP, C), dtype=bfnp)
        p[:, 1:1 + H, 1:1 + W, :] = ft.transpose(0, 2, 3, 1).astype(bfnp)
        p = p.reshape(B, GR, C)
        return np.ascontiguousarray(
            np.concatenate([p[:, :-1, :], p[:, 1:, :]], axis=2))

    fp2p = padgrid(ft_2)
    fp3p = padgrid(ft_3)

    in_maps = []
    for core in range(NCORES):
        b, s = divmod(core, 4)
        r0 = s * SR
        xin = np.zeros((6, P, 38, WP), np.float32)
        lo, hi = r0 - 3, r0 + SR + 3            # conv1 input rows
        vlo, vhi = max(lo, 0), min(hi, H)
        xin[:, :, vlo - lo:vhi - lo, XOFF:XOFF + W] = (
            combined[b, :, vlo:vhi, :].reshape(6, P, vhi - vlo, W))
        ft1s = np.ascontiguousarray(
            ft_1[b, :, r0:r0 + SR, :].reshape(2, P, SR * W)).astype(bfnp)
        mt = np.zeros((P, 2), np.float32)
        mt[:, 1] = r0

        def bmask(rows, rlo):
            m = np.zeros((rows, WP), np.float32)
            for i in range(rows):
                if 0 <= rlo + i < H:
                    m[i, XOFF:XOFF + W] = 1.0
            return np.ascontiguousarray(
                np.broadcast_to(m.reshape(1, rows * WP), (P, rows * WP)))

        m1 = bmask(36, r0 - 2)
        m2 = bmask(34, r0 - 1)
        xinf = np.zeros((6, P, 38 * WP + 2), np.float32)
        xinf[:, :, 1:1 + 38 * WP] = xin.reshape(6, P, 38 * WP)
        fp2c = fp2p[b]
        fp3c = fp3p[b]
        in_maps.append({
            "xin": xinf,
            "ft1s": ft1s, "fp2p": fp2c, "fp3p": fp3c,
            "w1l": w1l, "w2l": w2l, "w3l": w3l,
            "wd2l": wd2l, "wd3l": wd3l, "wfl": wfl,
            "b1": b1p, "b2": b2p, "b3": b3p, "bf": bfp,
            "kyrow": kyrow, "xkx": xkx, "meta": mt, "ident": ident,
            "identb": identb,
            "m1m": m1, "m2m": m2,
        })
    return in_maps


def assemble_output(results):
    full = np.empty((B, C, H, W), np.float32)
    for core in range(NCORES):
        b, s = divmod(core, 4)
        r0 = s * SR
        o = results[core]["out"]            # [2, 128, SR*W]
        for oj in range(2):
            full[b, oj * P:(oj + 1) * P, r0:r0 + SR, :] = o[oj].reshape(
                P, SR, W)
    return full


_CACHED_NC = None


def kernel(**inputs) -> np.ndarray:
    global _CACHED_NC
    in_maps = prep_inputs(**inputs)
    if _CACHED_NC is None:
        _CACHED_NC = build_program()
    res = run_bass_kernel_spmd(_CACHED_NC, in_maps, list(range(NCORES)))
    return assemble_output(res.results)


if __name__ == "__main__":
    import json
    rng = np.random.default_rng(0)
    print("building program (syntax check)...")
    nc = build_program()
    print("instructions:",
          sum(len(bb.instructions) for bb in nc.m.functions[0].blocks))
